# revision 8
# baseline (speedup 1.0000x reference)
"""Trainium2 Bass kernel for a GQA causal-attention block (TP over heads, 8 cores).

Computation (per reference): q/k/v projections of x, interleaved RoPE on q/k,
GQA causal attention (32 q heads, 8 kv heads, head_dim 128, seq 2048), output
projection. Sharding: tensor-parallel over heads — each core owns 4 q heads and
their shared kv head. The attention output (transposed layout) is AllGathered
across cores per 512-sequence window and each core computes a 512-column slice
of the final output projection; the host concatenates the column slices.

Device dataflow notes:
  - Everything transposed: x^T streams as the matmul moving operand so q^T/k^T
    come out with head_dim on partitions; scores are computed transposed
    (s^T[k_pos, q_pos]) so exp(s^T) feeds the PV matmul directly as the moving
    operand without any on-chip transposes.
  - RoPE uses a half-split head_dim permutation (evens then odds), folded into
    the wq/wk columns on the host, so the rotation is two 64-partition
    multiply/add pairs against host-transposed cos/sin tables.
  - Softmax skips the max subtraction (scores ~ N(0,1) after scaling); row sums
    come from a ones-matmul accumulated alongside PV; normalization is folded
    into the psum->sbuf eviction of the attention output via a PE-broadcast
    reciprocal row.
  - Causal masking multiplies exp(scores) by a 0/1 mask on diagonal blocks only.
  - Window 0 of the QKV phase runs contraction-outer so the PE streams behind
    the initial DMAs instead of stalling on the full window load.
  - The AllGather is split per 512-seq window; each window's output-projection
    slice is emitted one window later, so collectives and gathered-activation
    DMAs always have a full attention window of PE work to hide behind.
"""

import numpy as np
import ml_dtypes

import concourse.bass as bass
import concourse.mybir as mybir
import concourse.tile as tile
from concourse import bacc
from concourse.bass_utils import run_bass_kernel_spmd

N_CORES = 8
P = 128
SEQ = 2048
DIM = 4096
N_HEADS = 32
N_KV_HEADS = 8
HD = 128
QH = N_HEADS // N_CORES        # q heads per core
KD = DIM // P                  # contraction chunks
KG = 4                         # k-chunk DMA groups
KPG = KD // KG                 # k chunks per group
W = 512                        # seq window (matmul moving free dim)
NW = SEQ // W
NT = SEQ // P
OUTC = DIM // N_CORES          # output columns per core
SCALE = HD ** -0.5

BF16 = mybir.dt.bfloat16
F32 = mybir.dt.float32


def _build_nc():
    nc = bacc.Bacc("TRN2", target_bir_lowering=False, debug=False,
                   num_devices=N_CORES)

    xt_d = nc.dram_tensor("xt", [DIM, SEQ], BF16, kind="ExternalInput")
    wq_d = nc.dram_tensor("wq", [DIM, QH * HD], BF16, kind="ExternalInput")
    wk_d = nc.dram_tensor("wk", [DIM, HD], BF16, kind="ExternalInput")
    wv_d = nc.dram_tensor("wv", [DIM, HD], BF16, kind="ExternalInput")
    wo_d = nc.dram_tensor("wo", [DIM, OUTC], BF16, kind="ExternalInput")
    cs_d = nc.dram_tensor("cs", [64, SEQ], F32, kind="ExternalInput")
    sn_d = nc.dram_tensor("sn", [64, SEQ], F32, kind="ExternalInput")
    out_d = nc.dram_tensor("out", [SEQ, OUTC], F32, kind="ExternalOutput")

    # 0/1 causal masks for the 4 diagonal alignments of a [128 kv, 512 q] block:
    # mask[p, c, q] = 1 iff kv offset p + c*128 <= q (within the 512-q window).
    j = np.arange(P)[:, None, None]
    c = np.arange(4)[None, :, None]
    q = np.arange(W)[None, None, :]
    masks_np = (j + c * P <= q).astype(ml_dtypes.bfloat16)
    masks_d = nc.inline_tensor(masks_np, "cmasks")
    ones_d = nc.inline_tensor(np.ones([P, P], dtype=ml_dtypes.bfloat16), "ones")
    ones1f_d = nc.inline_tensor(np.ones([1, P], dtype=np.float32), "ones1f")

    Exp = mybir.ActivationFunctionType.Exp

    with tile.TileContext(nc) as tc:
        with tc.tile_pool(name="const", bufs=1) as constp, \
             tc.tile_pool(name="acts", bufs=1) as actp:
            qrot = actp.tile([P, QH, SEQ], BF16)   # q^T (rope'd), per head
            krot = actp.tile([P, SEQ], BF16)       # k^T (rope'd)
            v_sb = actp.tile([P, NT, HD], BF16)    # v row-tiles [seq, hd]

            # ---- Phase 1: QKV projections + RoPE ----
            with tc.tile_pool(name="wqkv", bufs=1) as wp, \
                 tc.tile_pool(name="xtp", bufs=2) as xp, \
                 tc.tile_pool(name="ropet", bufs=2) as rtp:
                # interleave per-group loads so compute can stream behind DMA
                wq_g = [wp.tile([P, KPG, QH * HD], BF16, name=f"wq{g}")
                        for g in range(KG)]
                wk_g = [wp.tile([P, KPG, HD], BF16, name=f"wk{g}")
                        for g in range(KG)]
                wv_g = [wp.tile([P, KPG, HD], BF16, name=f"wv{g}")
                        for g in range(KG)]
                xt_gs: dict[tuple[int, int], bass.AP] = {}

                def load_xt_group(w, g):
                    t = xp.tile([P, KPG, W], BF16, tag=f"xt{g}")
                    k0 = g * KPG
                    nc.sync.dma_start(
                        out=t[:],
                        in_=xt_d[k0 * P:(k0 + KPG) * P, w * W:(w + 1) * W]
                        .rearrange("(k p) n -> p k n", p=P))
                    xt_gs[(w, g)] = t

                def grp(handle, g):
                    return handle[g * KPG * P:(g + 1) * KPG * P, :] \
                        .rearrange("(k p) n -> p k n", p=P)

                for g in range(KG):
                    load_xt_group(0, g)
                    nc.sync.dma_start(out=wq_g[g][:], in_=grp(wq_d, g))
                    nc.sync.dma_start(out=wk_g[g][:], in_=grp(wk_d, g))
                    nc.sync.dma_start(out=wv_g[g][:], in_=grp(wv_d, g))
                    if g == 0:
                        cs_sb = constp.tile([64, SEQ], F32)
                        nc.sync.dma_start(out=cs_sb[:], in_=cs_d[:])
                        sn_sb = constp.tile([64, SEQ], F32)
                        nc.sync.dma_start(out=sn_sb[:], in_=sn_d[:])
                masks_sb = constp.tile([P, 4, W], BF16)
                nc.sync.dma_start(out=masks_sb[:], in_=masks_d[:])
                ones_sb = constp.tile([P, P], BF16)
                nc.sync.dma_start(out=ones_sb[:], in_=ones_d[:])
                ones1f_sb = constp.tile([1, P], F32)
                nc.sync.dma_start(out=ones1f_sb[:], in_=ones1f_d[:])

                def rope(ps, dst, s0):
                    cw = cs_sb[:, s0:s0 + W]
                    sw = sn_sb[:, s0:s0 + W]
                    t1 = rtp.tile([64, W], F32, tag="t1")
                    t2 = rtp.tile([64, W], F32, tag="t2")
                    # even half: x1*cos - x2*sin
                    nc.vector.tensor_mul(t1[:], ps[0:64, :], cw)
                    nc.vector.tensor_mul(t2[:], ps[64:128, :], sw)
                    nc.vector.tensor_sub(dst[0:64], t1[:], t2[:])
                    # odd half: x1*sin + x2*cos
                    t3 = rtp.tile([64, W], F32, tag="t3")
                    t4 = rtp.tile([64, W], F32, tag="t4")
                    nc.vector.tensor_mul(t3[:], ps[0:64, :], sw)
                    nc.vector.tensor_mul(t4[:], ps[64:128, :], cw)
                    nc.vector.tensor_add(dst[64:128], t3[:], t4[:])

                # window 0: contraction-outer (q/k heads) so the PE streams
                # behind the initial DMAs. Each psum tile owns a full bank —
                # start=True zeroes a whole 2KB zero-region, so accumulation
                # groups must never share a bank.
                with tc.tile_pool(name="ps0", bufs=1, space="PSUM") as p0:
                    ps_w0 = [p0.tile([P, W], F32, name=f"ps0_{m}")
                             for m in range(QH + 1)]
                    for k in range(KD):
                        g, kk = k // KPG, k % KPG
                        xg = xt_gs[(0, g)]
                        st, sp = (k == 0), (k == KD - 1)
                        for m in range(QH):
                            nc.tensor.matmul(ps_w0[m][:],
                                             wq_g[g][:, kk, m * HD:(m + 1) * HD],
                                             xg[:, kk, :], start=st, stop=sp)
                        nc.tensor.matmul(ps_w0[QH][:], wk_g[g][:, kk, :],
                                         xg[:, kk, :], start=st, stop=sp)
                    for g in range(KG):
                        load_xt_group(1, g)
                    for m in range(QH):
                        rope(ps_w0[m], qrot[:, m, 0:W], 0)
                    rope(ps_w0[QH], krot[:, 0:W], 0)

                # remaining windows head-outer; window 0's v tiles first (their
                # xt groups are already resident, so no DMA stall)
                with tc.tile_pool(name="psq", bufs=2, space="PSUM") as pq, \
                     tc.tile_pool(name="psv", bufs=2, space="PSUM") as pv:
                    for mm in range(W // P):
                        psv = pv.tile([P, HD], F32, tag="psv")
                        for k in range(KD):
                            g, kk = k // KPG, k % KPG
                            nc.tensor.matmul(
                                psv[:],
                                xt_gs[(0, g)][:, kk, mm * P:(mm + 1) * P],
                                wv_g[g][:, kk, :],
                                start=(k == 0), stop=(k == KD - 1))
                        nc.scalar.copy(v_sb[:, mm, :], psv[:])
                    for w in range(1, NW):
                        s0 = w * W
                        if w + 1 < NW:
                            for g in range(KG):
                                load_xt_group(w + 1, g)
                        for m in range(QH + 1):
                            ps = pq.tile([P, W], F32, tag="psq")
                            for k in range(KD):
                                g, kk = k // KPG, k % KPG
                                lhsT = (wq_g[g][:, kk, m * HD:(m + 1) * HD]
                                        if m < QH else wk_g[g][:, kk, :])
                                nc.tensor.matmul(ps[:], lhsT,
                                                 xt_gs[(w, g)][:, kk, :],
                                                 start=(k == 0), stop=(k == KD - 1))
                            dst = (qrot[:, m, s0:s0 + W] if m < QH
                                   else krot[:, s0:s0 + W])
                            rope(ps, dst, s0)
                        for mm in range(W // P):
                            psv = pv.tile([P, HD], F32, tag="psv")
                            for k in range(KD):
                                g, kk = k // KPG, k % KPG
                                nc.tensor.matmul(
                                    psv[:],
                                    xt_gs[(w, g)][:, kk, mm * P:(mm + 1) * P],
                                    wv_g[g][:, kk, :],
                                    start=(k == 0), stop=(k == KD - 1))
                            nc.scalar.copy(v_sb[:, w * (W // P) + mm, :], psv[:])

            # ---- Phase 2+3: attention, windowed AllGather, output proj ----
            with tc.tile_pool(name="ccdram", bufs=1, space="DRAM") as dcc, \
                 tc.tile_pool(name="wop", bufs=1) as wop, \
                 tc.tile_pool(name="pss", bufs=2, space="PSUM") as psp, \
                 tc.tile_pool(name="pso", bufs=2, space="PSUM") as pso, \
                 tc.tile_pool(name="pssum", bufs=2, space="PSUM") as pss, \
                 tc.tile_pool(name="psw", bufs=2, space="PSUM") as psw, \
                 tc.tile_pool(name="ptp", bufs=4) as ptp, \
                 tc.tile_pool(name="attp", bufs=2) as attp, \
                 tc.tile_pool(name="atfp", bufs=2) as atfp, \
                 tc.tile_pool(name="outp", bufs=2) as outp:
                wo_sb = wop.tile([P, KD, OUTC], BF16)
                nc.sync.dma_start(out=wo_sb[:],
                                  in_=wo_d[:].rearrange("(k p) n -> p k n", p=P))
                ag_in = [dcc.tile([QH * HD, W], BF16, name=f"agin{w}")
                         for w in range(NW)]
                ag_out = [dcc.tile([DIM, W], BF16, addr_space="Shared",
                                   name=f"agout{w}")
                          for w in range(NW)]

                def emit_attention(qc):
                    q0 = qc * W
                    nkv = (W // P) * (qc + 1)
                    for h in range(QH):
                        ps_o = pso.tile([P, W], F32, tag="o")
                        ps_sum = pss.tile([P, W], F32, tag="sum")
                        for jj in range(nkv):
                            j0 = jj * P
                            ps_s = psp.tile([P, W], F32, tag="s")
                            nc.tensor.matmul(ps_s[:], krot[:, j0:j0 + P],
                                             qrot[:, h, q0:q0 + W],
                                             start=True, stop=True)
                            pt = ptp.tile([P, W], BF16, tag="pt")
                            nc.scalar.activation(pt[:], ps_s[:], Exp, scale=SCALE)
                            if jj >= (W // P) * qc:
                                nc.vector.tensor_mul(
                                    pt[:], pt[:],
                                    masks_sb[:, jj - (W // P) * qc, :])
                            nc.tensor.matmul(ps_o[:], v_sb[:, jj, :], pt[:],
                                             start=(jj == 0), stop=(jj == nkv - 1))
                            nc.tensor.matmul(ps_sum[:], ones_sb[:], pt[:],
                                             start=(jj == 0), stop=(jj == nkv - 1))
                        # normalization: reciprocal of one row, PE-broadcast to
                        # 128 partitions, multiply into the psum eviction.
                        inv1 = attp.tile([1, W], F32, tag="inv1")
                        nc.vector.reciprocal(inv1[:], ps_sum[0:1, :])
                        ps_bc = psp.tile([P, W], F32, tag="s")
                        nc.tensor.matmul(ps_bc[:], ones1f_sb[:], inv1[:],
                                         start=True, stop=True)
                        inv_bc = attp.tile([P, W], F32, tag="invbc")
                        nc.vector.tensor_copy(inv_bc[:], ps_bc[:])
                        at = attp.tile([P, W], BF16, tag="at")
                        nc.vector.tensor_mul(at[:], ps_o[:], inv_bc[:])
                        nc.sync.dma_start(out=ag_in[qc][h * HD:(h + 1) * HD, :],
                                          in_=at[:])
                    nc.gpsimd.collective_compute(
                        "AllGather", mybir.AluOpType.bypass,
                        replica_groups=[list(range(N_CORES))],
                        ins=[ag_in[qc][:]], outs=[ag_out[qc][:]])

                def emit_wo(qc):
                    q0 = qc * W
                    atf = atfp.tile([P, KD, W], BF16, tag="atf")
                    nc.sync.dma_start(
                        out=atf[:],
                        in_=ag_out[qc][:].rearrange("(k p) n -> p k n", p=P))
                    for mm in range(W // P):
                        m0 = mm * P
                        ps = psw.tile([P, OUTC], F32, tag="w")
                        for k in range(KD):
                            nc.tensor.matmul(ps[:],
                                             atf[:, k, m0:m0 + P],
                                             wo_sb[:, k, :],
                                             start=(k == 0), stop=(k == KD - 1))
                        ot = outp.tile([P, OUTC], F32, tag="ot")
                        nc.vector.tensor_copy(ot[:], ps[:])
                        nc.sync.dma_start(out=out_d[q0 + m0:q0 + m0 + P, :],
                                          in_=ot[:])

                for qc in range(NW):
                    emit_attention(qc)
                    if qc >= 1:
                        emit_wo(qc - 1)
                emit_wo(NW - 1)

    nc.compile()
    return nc


_NC_CACHE = None


def _get_nc():
    global _NC_CACHE
    if _NC_CACHE is None:
        _NC_CACHE = _build_nc()
    return _NC_CACHE


def make_in_maps(x, freqs_cos, freqs_sin, wq, wk, wv, wo):
    bf16 = ml_dtypes.bfloat16
    # half-split permutation: evens then odds within each head's 128 dims
    pidx = np.concatenate([np.arange(0, HD, 2), np.arange(1, HD, 2)])
    xt = np.ascontiguousarray(np.asarray(x).T.astype(bf16))
    cs = np.ascontiguousarray(np.asarray(freqs_cos).T.astype(np.float32))
    sn = np.ascontiguousarray(np.asarray(freqs_sin).T.astype(np.float32))
    wq = np.asarray(wq)
    wk = np.asarray(wk)
    wv = np.asarray(wv)
    wo = np.asarray(wo)
    in_maps = []
    for core in range(N_CORES):
        q_cols = np.concatenate([h * HD + pidx
                                 for h in range(QH * core, QH * (core + 1))])
        in_maps.append({
            "xt": xt,
            "wq": np.ascontiguousarray(wq[:, q_cols].astype(bf16)),
            "wk": np.ascontiguousarray(wk[:, core * HD + pidx].astype(bf16)),
            "wv": np.ascontiguousarray(wv[:, core * HD:(core + 1) * HD].astype(bf16)),
            "wo": np.ascontiguousarray(wo[:, core * OUTC:(core + 1) * OUTC].astype(bf16)),
            "cs": cs,
            "sn": sn,
        })
    return in_maps


def kernel(x, freqs_cos, freqs_sin, wq, wk, wv, wo, _run_kwargs=None):
    in_maps = make_in_maps(x, freqs_cos, freqs_sin, wq, wk, wv, wo)
    nc = _get_nc()
    res = run_bass_kernel_spmd(nc, in_maps, list(range(N_CORES)),
                               **(_run_kwargs or {}))
    out = np.concatenate([res.results[i]["out"] for i in range(N_CORES)], axis=1)
    if _run_kwargs is not None:
        kernel.last_results = res
    return np.ascontiguousarray(out.astype(np.float32))


# revision 12
# speedup vs baseline: 1.1228x; 1.1228x over previous
"""Trainium2 Bass kernel for a GQA causal-attention block (TP over heads, 8 cores).

Computation (per reference): q/k/v projections of x, interleaved RoPE on q/k,
GQA causal attention (32 q heads, 8 kv heads, head_dim 128, seq 2048), output
projection. Sharding: tensor-parallel over heads — each core owns 4 q heads and
their shared kv head. The attention output (transposed layout) is AllGathered
across cores per 512-sequence window and each core computes a 512-column slice
of the final output projection; the host concatenates the column slices.

Device dataflow notes:
  - Everything transposed: x^T streams as the matmul moving operand so q^T/k^T
    come out with head_dim on partitions; scores are computed transposed
    (s^T[k_pos, q_pos]) so exp(s^T) feeds the PV matmul directly as the moving
    operand without any on-chip transposes.
  - RoPE uses a half-split head_dim permutation (evens then odds), folded into
    the wq/wk columns on the host, so the rotation is two 64-partition
    multiply/add pairs against host-transposed cos/sin tables.
  - Softmax skips the max subtraction (scores ~ N(0,1) after scaling); row sums
    come from a ones-matmul accumulated alongside PV; normalization is folded
    into the psum->sbuf eviction of the attention output via a PE-broadcast
    reciprocal row.
  - Causal masking multiplies exp(scores) by a 0/1 mask on diagonal blocks only.
  - Global software pipeline keeps the PE stream dense (no HAM re-throttle):
    QKV(0) runs contraction-outer behind the initial DMAs, then attention
    window qc is emitted right after QKV(qc) (causality makes window qc fully
    computable then), AllGathers fire per window, and each window's output
    projection is emitted several phases later so collective + gather-DMA
    latency hides under guaranteed PE work. Attention windows run in order
    0,1,3,2 so the final output-projection slice has cover for its AllGather.
  - PSUM is exactly 8 banks: window-0 k-outer uses 5 transient banks; steady
    state shares 2 banks between QKV-psum/scores, 2 between v/wo-psum, plus
    2 PV accumulators and 2 softmax-sum accumulators.
"""

import numpy as np
import ml_dtypes

import concourse.bass as bass
import concourse.mybir as mybir
import concourse.tile as tile
from concourse import bacc
from concourse.bass_utils import run_bass_kernel_spmd

N_CORES = 8
P = 128
SEQ = 2048
DIM = 4096
N_HEADS = 32
N_KV_HEADS = 8
HD = 128
QH = N_HEADS // N_CORES        # q heads per core
KD = DIM // P                  # contraction chunks
KG = 4                         # k-chunk DMA groups
KPG = KD // KG                 # k chunks per group
W = 512                        # seq window (matmul moving free dim)
NW = SEQ // W
NT = SEQ // P
OUTC = DIM // N_CORES          # output columns per core
SCALE = HD ** -0.5

BF16 = mybir.dt.bfloat16
F32 = mybir.dt.float32


def _build_nc():
    nc = bacc.Bacc("TRN2", target_bir_lowering=False, debug=False,
                   num_devices=N_CORES)

    xt_d = nc.dram_tensor("xt", [DIM, SEQ], BF16, kind="ExternalInput")
    wq_d = nc.dram_tensor("wq", [DIM, QH * HD], BF16, kind="ExternalInput")
    wk_d = nc.dram_tensor("wk", [DIM, HD], BF16, kind="ExternalInput")
    wv_d = nc.dram_tensor("wv", [DIM, HD], BF16, kind="ExternalInput")
    wo_d = nc.dram_tensor("wo", [DIM, OUTC], BF16, kind="ExternalInput")
    cs_d = nc.dram_tensor("cs", [64, SEQ], F32, kind="ExternalInput")
    sn_d = nc.dram_tensor("sn", [64, SEQ], F32, kind="ExternalInput")
    out_d = nc.dram_tensor("out", [SEQ, OUTC], F32, kind="ExternalOutput")

    # 0/1 causal masks for the 4 diagonal alignments of a [128 kv, 512 q] block:
    # mask[p, c, q] = 1 iff kv offset p + c*128 <= q (within the 512-q window).
    j = np.arange(P)[:, None, None]
    c = np.arange(4)[None, :, None]
    q = np.arange(W)[None, None, :]
    masks_np = (j + c * P <= q).astype(ml_dtypes.bfloat16)
    masks_d = nc.inline_tensor(masks_np, "cmasks")
    ones_d = nc.inline_tensor(np.ones([P, P], dtype=ml_dtypes.bfloat16), "ones")
    ones1f_d = nc.inline_tensor(np.ones([1, P], dtype=np.float32), "ones1f")

    Exp = mybir.ActivationFunctionType.Exp

    with tile.TileContext(nc) as tc:
        with tc.tile_pool(name="const", bufs=1) as constp, \
             tc.tile_pool(name="acts", bufs=1) as actp, \
             tc.tile_pool(name="wqkv", bufs=1) as wp, \
             tc.tile_pool(name="xtp", bufs=2) as xp, \
             tc.tile_pool(name="ropet", bufs=2) as rtp, \
             tc.tile_pool(name="ccdram", bufs=1, space="DRAM") as dcc, \
             tc.tile_pool(name="wop", bufs=1) as wop:
            qrot = actp.tile([P, QH, SEQ], BF16)   # q^T (rope'd), per head
            krot = actp.tile([P, SEQ], BF16)       # k^T (rope'd)
            v_sb = actp.tile([P, NT, HD], BF16)    # v row-tiles [seq, hd]

            wq_g = [wp.tile([P, KPG, QH * HD], BF16, name=f"wq{g}")
                    for g in range(KG)]
            wk_g = [wp.tile([P, KPG, HD], BF16, name=f"wk{g}")
                    for g in range(KG)]
            wv_g = [wp.tile([P, KPG, HD], BF16, name=f"wv{g}")
                    for g in range(KG)]
            xt_gs: dict[tuple[int, int], bass.AP] = {}

            def load_xt_group(w, g):
                t = xp.tile([P, KPG, W], BF16, tag=f"xt{g}", bufs=2 if g < 2 else 1)
                k0 = g * KPG
                nc.sync.dma_start(
                    out=t[:],
                    in_=xt_d[k0 * P:(k0 + KPG) * P, w * W:(w + 1) * W]
                    .rearrange("(k p) n -> p k n", p=P))
                xt_gs[(w, g)] = t

            def grp(handle, g):
                return handle[g * KPG * P:(g + 1) * KPG * P, :] \
                    .rearrange("(k p) n -> p k n", p=P)

            # initial DMA order: window-0 groups interleaved with weights, then
            # window 1, with the small tables slotted where they're needed.
            for g in range(KG):
                load_xt_group(0, g)
                nc.sync.dma_start(out=wq_g[g][:], in_=grp(wq_d, g))
                nc.sync.dma_start(out=wk_g[g][:], in_=grp(wk_d, g))
                nc.sync.dma_start(out=wv_g[g][:], in_=grp(wv_d, g))
            load_xt_group(1, 0)
            load_xt_group(1, 1)
            trig_sb = constp.tile([P, SEQ], F32)
            nc.sync.dma_start(out=trig_sb[0:64, :], in_=cs_d[:])
            nc.sync.dma_start(out=trig_sb[64:128, :], in_=sn_d[:])
            load_xt_group(1, 2)
            load_xt_group(1, 3)
            masks_sb = constp.tile([P, 4, W], BF16)
            nc.sync.dma_start(out=masks_sb[:], in_=masks_d[:])
            ones_sb = constp.tile([P, P], BF16)
            nc.sync.dma_start(out=ones_sb[:], in_=ones_d[:])
            ones1f_sb = constp.tile([1, P], F32)
            nc.sync.dma_start(out=ones1f_sb[:], in_=ones1f_d[:])
            wo_sb = wop.tile([P, KD, OUTC], BF16)
            nc.sync.dma_start(out=wo_sb[:],
                              in_=wo_d[:].rearrange("(k p) n -> p k n", p=P))

            ag_in = [dcc.tile([QH * HD, W], BF16, name=f"agin{w}")
                     for w in range(NW)]
            ag_out = [dcc.tile([DIM, W], BF16, addr_space="Shared",
                               name=f"agout{w}")
                      for w in range(NW)]

            def rope(ps, dst, s0):
                cw = trig_sb[0:64, s0:s0 + W]
                sw = trig_sb[64:128, s0:s0 + W]
                t1 = rtp.tile([64, W], F32, tag="t1")
                t2 = rtp.tile([64, W], F32, tag="t2")
                # even half: x1*cos - x2*sin
                nc.vector.tensor_mul(t1[:], ps[0:64, :], cw)
                nc.vector.tensor_mul(t2[:], ps[64:128, :], sw)
                nc.vector.tensor_sub(dst[0:64], t1[:], t2[:])
                # odd half: x1*sin + x2*cos (reuse the t1/t2 slots)
                t3 = rtp.tile([64, W], F32, tag="t1")
                t4 = rtp.tile([64, W], F32, tag="t2")
                nc.vector.tensor_mul(t3[:], ps[0:64, :], sw)
                nc.vector.tensor_mul(t4[:], ps[64:128, :], cw)
                nc.vector.tensor_add(dst[64:128], t3[:], t4[:])

            # window 0 q/k: contraction-outer so PE streams behind the DMAs.
            # Each psum tile owns a full bank (start=True zeroes a whole 2KB
            # zero-region, so accumulation groups must never share a bank).
            with tc.tile_pool(name="ps0", bufs=1, space="PSUM") as p0:
                ps_w0 = [p0.tile([P, W], F32, name=f"ps0_{m}")
                         for m in range(QH + 1)]
                for k in range(KD):
                    g, kk = k // KPG, k % KPG
                    xg = xt_gs[(0, g)]
                    st, sp = (k == 0), (k == KD - 1)
                    for m in range(QH):
                        nc.tensor.matmul(ps_w0[m][:],
                                         wq_g[g][:, kk, m * HD:(m + 1) * HD],
                                         xg[:, kk, :], start=st, stop=sp)
                    nc.tensor.matmul(ps_w0[QH][:], wk_g[g][:, kk, :],
                                     xg[:, kk, :], start=st, stop=sp)
                for m in range(QH):
                    rope(ps_w0[m], qrot[:, m, 0:W], 0)
                rope(ps_w0[QH], krot[:, 0:W], 0)

            # steady-state psum pools: exactly 8 banks, with QKV/scores and
            # v/wo sharing slots (the streams interleave on the PE anyway).
            with tc.tile_pool(name="pmixs", bufs=2, space="PSUM") as pmixs, \
                 tc.tile_pool(name="pmixv", bufs=2, space="PSUM") as pmixv, \
                 tc.tile_pool(name="pso", bufs=2, space="PSUM") as pso, \
                 tc.tile_pool(name="pssum", bufs=2, space="PSUM") as pss, \
                 tc.tile_pool(name="ptp", bufs=3) as ptp, \
                 tc.tile_pool(name="attp", bufs=2) as attp, \
                 tc.tile_pool(name="atfp", bufs=2) as atfp, \
                 tc.tile_pool(name="outp", bufs=2) as outp:

                def emit_v(w):
                    for mm in range(W // P):
                        psv = pmixv.tile([P, OUTC], F32, tag="vw")
                        for k in range(KD):
                            g, kk = k // KPG, k % KPG
                            nc.tensor.matmul(
                                psv[:, 0:HD],
                                xt_gs[(w, g)][:, kk, mm * P:(mm + 1) * P],
                                wv_g[g][:, kk, :],
                                start=(k == 0), stop=(k == KD - 1))
                        nc.scalar.copy(v_sb[:, w * (W // P) + mm, :],
                                       psv[:, 0:HD])

                def emit_qkv(w):
                    s0 = w * W
                    if w + 1 < NW:
                        for g in range(KG):
                            load_xt_group(w + 1, g)
                    for m in range(QH + 1):
                        ps = pmixs.tile([P, W], F32, tag="s")
                        for k in range(KD):
                            g, kk = k // KPG, k % KPG
                            lhsT = (wq_g[g][:, kk, m * HD:(m + 1) * HD]
                                    if m < QH else wk_g[g][:, kk, :])
                            nc.tensor.matmul(ps[:], lhsT,
                                             xt_gs[(w, g)][:, kk, :],
                                             start=(k == 0), stop=(k == KD - 1))
                        dst = (qrot[:, m, s0:s0 + W] if m < QH
                               else krot[:, s0:s0 + W])
                        rope(ps, dst, s0)
                    emit_v(w)

                def emit_attention(qc):
                    q0 = qc * W
                    nkv = (W // P) * (qc + 1)
                    for h in range(QH):
                        ps_o = pso.tile([P, W], F32, tag="o")
                        ps_sum = pss.tile([P, W], F32, tag="sum")
                        for jj in range(nkv):
                            j0 = jj * P
                            ps_s = pmixs.tile([P, W], F32, tag="s")
                            nc.tensor.matmul(ps_s[:], krot[:, j0:j0 + P],
                                             qrot[:, h, q0:q0 + W],
                                             start=True, stop=True)
                            pt = ptp.tile([P, W], BF16, tag="pt")
                            nc.scalar.activation(pt[:], ps_s[:], Exp, scale=SCALE)
                            if jj >= (W // P) * qc:
                                nc.vector.tensor_mul(
                                    pt[:], pt[:],
                                    masks_sb[:, jj - (W // P) * qc, :])
                            nc.tensor.matmul(ps_o[:], v_sb[:, jj, :], pt[:],
                                             start=(jj == 0), stop=(jj == nkv - 1))
                            nc.tensor.matmul(ps_sum[:], ones_sb[:], pt[:],
                                             start=(jj == 0), stop=(jj == nkv - 1))
                        # normalization: reciprocal of one row, PE-broadcast to
                        # 128 partitions, multiply into the psum eviction.
                        inv1 = attp.tile([1, W], F32, tag="inv1")
                        nc.vector.reciprocal(inv1[:], ps_sum[0:1, :])
                        ps_bc = pmixs.tile([P, W], F32, tag="s")
                        nc.tensor.matmul(ps_bc[:], ones1f_sb[:], inv1[:],
                                         start=True, stop=True)
                        inv_bc = attp.tile([P, W], F32, tag="invbc")
                        nc.vector.tensor_copy(inv_bc[:], ps_bc[:])
                        at = attp.tile([P, W], BF16, tag="at")
                        nc.vector.tensor_mul(at[:], ps_o[:], inv_bc[:])
                        nc.sync.dma_start(out=ag_in[qc][h * HD:(h + 1) * HD, :],
                                          in_=at[:])
                    nc.gpsimd.collective_compute(
                        "AllGather", mybir.AluOpType.bypass,
                        replica_groups=[list(range(N_CORES))],
                        ins=[ag_in[qc][:]], outs=[ag_out[qc][:]])

                def emit_wo(qc):
                    q0 = qc * W
                    for mm in range(W // P):
                        m0 = mm * P
                        atf = atfp.tile([P, KD, P], BF16, tag="atf")
                        nc.sync.dma_start(
                            out=atf[:],
                            in_=ag_out[qc][:, m0:m0 + P]
                            .rearrange("(k p) n -> p k n", p=P))
                        ps = pmixv.tile([P, OUTC], F32, tag="vw")
                        for k in range(KD):
                            nc.tensor.matmul(ps[:],
                                             atf[:, k, :],
                                             wo_sb[:, k, :],
                                             start=(k == 0), stop=(k == KD - 1))
                        ot = outp.tile([P, OUTC], F32, tag="ot")
                        nc.vector.tensor_copy(ot[:], ps[:])
                        nc.sync.dma_start(out=out_d[q0 + m0:q0 + m0 + P, :],
                                          in_=ot[:])

                emit_v(0)
                emit_attention(0)
                emit_qkv(1)
                emit_attention(1)
                emit_qkv(2)
                emit_qkv(3)
                emit_attention(3)
                emit_wo(0)
                emit_attention(2)
                emit_wo(1)
                emit_wo(3)
                emit_wo(2)

    nc.compile()
    return nc


_NC_CACHE = None


def _get_nc():
    global _NC_CACHE
    if _NC_CACHE is None:
        _NC_CACHE = _build_nc()
    return _NC_CACHE


def make_in_maps(x, freqs_cos, freqs_sin, wq, wk, wv, wo):
    bf16 = ml_dtypes.bfloat16
    # half-split permutation: evens then odds within each head's 128 dims
    pidx = np.concatenate([np.arange(0, HD, 2), np.arange(1, HD, 2)])
    xt = np.ascontiguousarray(np.asarray(x).T.astype(bf16))
    cs = np.ascontiguousarray(np.asarray(freqs_cos).T.astype(np.float32))
    sn = np.ascontiguousarray(np.asarray(freqs_sin).T.astype(np.float32))
    wq = np.asarray(wq)
    wk = np.asarray(wk)
    wv = np.asarray(wv)
    wo = np.asarray(wo)
    in_maps = []
    for core in range(N_CORES):
        q_cols = np.concatenate([h * HD + pidx
                                 for h in range(QH * core, QH * (core + 1))])
        in_maps.append({
            "xt": xt,
            "wq": np.ascontiguousarray(wq[:, q_cols].astype(bf16)),
            "wk": np.ascontiguousarray(wk[:, core * HD + pidx].astype(bf16)),
            "wv": np.ascontiguousarray(wv[:, core * HD:(core + 1) * HD].astype(bf16)),
            "wo": np.ascontiguousarray(wo[:, core * OUTC:(core + 1) * OUTC].astype(bf16)),
            "cs": cs,
            "sn": sn,
        })
    return in_maps


def kernel(x, freqs_cos, freqs_sin, wq, wk, wv, wo, _run_kwargs=None):
    in_maps = make_in_maps(x, freqs_cos, freqs_sin, wq, wk, wv, wo)
    nc = _get_nc()
    res = run_bass_kernel_spmd(nc, in_maps, list(range(N_CORES)),
                               **(_run_kwargs or {}))
    out = np.concatenate([res.results[i]["out"] for i in range(N_CORES)], axis=1)
    if _run_kwargs is not None:
        kernel.last_results = res
    return np.ascontiguousarray(out.astype(np.float32))


# revision 15
# speedup vs baseline: 1.1571x; 1.0305x over previous
"""Trainium2 Bass kernel for a GQA causal-attention block (TP over heads, 8 cores).

Computation (per reference): q/k/v projections of x, interleaved RoPE on q/k,
GQA causal attention (32 q heads, 8 kv heads, head_dim 128, seq 2048), output
projection. Sharding: tensor-parallel over heads — each core owns 4 q heads and
their shared kv head. The attention output (transposed layout) is AllGathered
across cores per 512-sequence window and each core computes a 512-column slice
of the final output projection; the host concatenates the column slices.

Device dataflow notes:
  - Everything transposed: x^T streams as the matmul moving operand so q^T/k^T
    come out with head_dim on partitions; scores are computed transposed
    (s^T[k_pos, q_pos]) so exp(s^T) feeds the PV matmul directly as the moving
    operand without any on-chip transposes.
  - RoPE uses a half-split head_dim permutation (evens then odds), folded into
    the wq/wk columns on the host, so the rotation is two 64-partition
    multiply/add pairs against host-transposed cos/sin tables.
  - Softmax skips the max subtraction (scores ~ N(0,1) after scaling); row sums
    come from a ones-matmul accumulated alongside PV; normalization is folded
    into the psum->sbuf eviction of the attention output via a PE-broadcast
    reciprocal row.
  - Causal masking multiplies exp(scores) by a 0/1 mask on diagonal blocks only.
  - Global software pipeline keeps the PE stream dense (no HAM re-throttle):
    QKV(0) runs contraction-outer behind the initial DMAs, then attention
    window qc is emitted right after QKV(qc) (causality makes window qc fully
    computable then), AllGathers fire per window, and each window's output
    projection is emitted several phases later so collective + gather-DMA
    latency hides under guaranteed PE work. Attention windows run in order
    0,1,3,2 so the final output-projection slice has cover for its AllGather.
  - PSUM is exactly 8 banks: window-0 k-outer uses 5 transient banks; steady
    state shares 2 banks between QKV-psum/scores, 2 between v/wo-psum, plus
    2 PV accumulators and 2 softmax-sum accumulators.
"""

import numpy as np
import ml_dtypes

import concourse.bass as bass
import concourse.mybir as mybir
import concourse.tile as tile
from concourse import bacc
from concourse.bass_utils import run_bass_kernel_spmd

N_CORES = 8
P = 128
SEQ = 2048
DIM = 4096
N_HEADS = 32
N_KV_HEADS = 8
HD = 128
QH = N_HEADS // N_CORES        # q heads per core
KD = DIM // P                  # contraction chunks
KG = 4                         # k-chunk DMA groups
KPG = KD // KG                 # k chunks per group
W = 512                        # seq window (matmul moving free dim)
NW = SEQ // W
NT = SEQ // P
OUTC = DIM // N_CORES          # output columns per core
SCALE = HD ** -0.5

BF16 = mybir.dt.bfloat16
F32 = mybir.dt.float32


def _build_nc():
    nc = bacc.Bacc("TRN2", target_bir_lowering=False, debug=False,
                   num_devices=N_CORES)

    xt_d = nc.dram_tensor("xt", [DIM, SEQ], BF16, kind="ExternalInput")
    wq_d = nc.dram_tensor("wq", [DIM, QH * HD], BF16, kind="ExternalInput")
    wk_d = nc.dram_tensor("wk", [DIM, HD], BF16, kind="ExternalInput")
    wv_d = nc.dram_tensor("wv", [DIM, HD], BF16, kind="ExternalInput")
    wo_d = nc.dram_tensor("wo", [DIM, OUTC], BF16, kind="ExternalInput")
    cs_d = nc.dram_tensor("cs", [64, SEQ], F32, kind="ExternalInput")
    sn_d = nc.dram_tensor("sn", [64, SEQ], F32, kind="ExternalInput")
    out_d = nc.dram_tensor("out", [SEQ, OUTC], F32, kind="ExternalOutput")

    # 0/1 causal masks for the 4 diagonal alignments of a [128 kv, 512 q] block:
    # mask[p, c, q] = 1 iff kv offset p + c*128 <= q (within the 512-q window).
    j = np.arange(P)[:, None, None]
    c = np.arange(4)[None, :, None]
    q = np.arange(W)[None, None, :]
    masks_np = (j + c * P <= q).astype(ml_dtypes.bfloat16)
    masks_d = nc.inline_tensor(masks_np, "cmasks")
    ones_d = nc.inline_tensor(np.ones([P, P], dtype=ml_dtypes.bfloat16), "ones")
    ones1f_d = nc.inline_tensor(np.ones([1, P], dtype=np.float32), "ones1f")

    Exp = mybir.ActivationFunctionType.Exp

    from contextlib import ExitStack
    with tile.TileContext(nc) as tc:
        with tc.tile_pool(name="const", bufs=1) as constp, \
             tc.tile_pool(name="acts", bufs=1) as actp, \
             tc.tile_pool(name="ropet", bufs=1) as rtp, \
             tc.tile_pool(name="ccdram", bufs=1, space="DRAM") as dcc, \
             tc.tile_pool(name="wop", bufs=1) as wop:
            region_a = ExitStack()
            wp = region_a.enter_context(tc.tile_pool(name="wqkv", bufs=1, side="right"))
            xp = region_a.enter_context(tc.tile_pool(name="xtp", bufs=2, side="right"))
            qrot = actp.tile([P, QH, SEQ], BF16)   # q^T (rope'd), per head
            krot = actp.tile([P, SEQ], BF16)       # k^T (rope'd)
            v_sb = actp.tile([P, NT, HD], BF16)    # v row-tiles [seq, hd]

            wq_g = [wp.tile([P, KPG, QH * HD], BF16, name=f"wq{g}")
                    for g in range(KG)]
            wk_g = [wp.tile([P, KPG, HD], BF16, name=f"wk{g}")
                    for g in range(KG)]
            wv_g = [wp.tile([P, KPG, HD], BF16, name=f"wv{g}")
                    for g in range(KG)]
            xt_gs: dict[tuple[int, int], bass.AP] = {}

            def load_xt_group(w, g):
                t = xp.tile([P, KPG, W], BF16, tag=f"xt{g}")
                k0 = g * KPG
                nc.sync.dma_start(
                    out=t[:],
                    in_=xt_d[k0 * P:(k0 + KPG) * P, w * W:(w + 1) * W]
                    .rearrange("(k p) n -> p k n", p=P))
                xt_gs[(w, g)] = t

            def grp(handle, g):
                return handle[g * KPG * P:(g + 1) * KPG * P, :] \
                    .rearrange("(k p) n -> p k n", p=P)

            # initial DMA order: window-0 groups interleaved with weights, then
            # window 1, with the small tables slotted where they're needed.
            for g in range(KG):
                load_xt_group(0, g)
                nc.sync.dma_start(out=wq_g[g][:], in_=grp(wq_d, g))
                nc.sync.dma_start(out=wk_g[g][:], in_=grp(wk_d, g))
                nc.sync.dma_start(out=wv_g[g][:], in_=grp(wv_d, g))
            load_xt_group(1, 0)
            load_xt_group(1, 1)
            trig_sb = constp.tile([P, SEQ], F32)
            nc.sync.dma_start(out=trig_sb[0:64, :], in_=cs_d[:])
            nc.sync.dma_start(out=trig_sb[64:128, :], in_=sn_d[:])
            load_xt_group(1, 2)
            load_xt_group(1, 3)
            masks_sb = constp.tile([P, 4, W], BF16)
            nc.sync.dma_start(out=masks_sb[:], in_=masks_d[:])
            ones_sb = constp.tile([P, P], BF16)
            nc.sync.dma_start(out=ones_sb[:], in_=ones_d[:])
            ones1f_sb = constp.tile([1, P], F32)
            nc.sync.dma_start(out=ones1f_sb[:], in_=ones1f_d[:])
            wo_sb = wop.tile([P, KD, OUTC], BF16)
            nc.sync.dma_start(out=wo_sb[:],
                              in_=wo_d[:].rearrange("(k p) n -> p k n", p=P))

            ag_in = [dcc.tile([QH * HD, W], BF16, name=f"agin{w}")
                     for w in range(NW)]
            ag_out = [dcc.tile([DIM, W], BF16, addr_space="Shared",
                               name=f"agout{w}")
                      for w in range(NW)]

            def rope(ps, dst, s0):
                cw = trig_sb[0:64, s0:s0 + W]
                sw = trig_sb[64:128, s0:s0 + W]
                t1 = rtp.tile([64, W], F32, tag="t1")
                t2 = rtp.tile([64, W], F32, tag="t2")
                # even half: x1*cos - x2*sin
                nc.vector.tensor_mul(t1[:], ps[0:64, :], cw)
                nc.vector.tensor_mul(t2[:], ps[64:128, :], sw)
                nc.vector.tensor_sub(dst[0:64], t1[:], t2[:])
                # odd half: x1*sin + x2*cos (reuse the t1/t2 slots)
                t3 = rtp.tile([64, W], F32, tag="t1")
                t4 = rtp.tile([64, W], F32, tag="t2")
                nc.vector.tensor_mul(t3[:], ps[0:64, :], sw)
                nc.vector.tensor_mul(t4[:], ps[64:128, :], cw)
                nc.vector.tensor_add(dst[64:128], t3[:], t4[:])

            # window 0 q/k: contraction-outer so PE streams behind the DMAs.
            # Each psum tile owns a full bank (start=True zeroes a whole 2KB
            # zero-region, so accumulation groups must never share a bank).
            with tc.tile_pool(name="ps0", bufs=1, space="PSUM") as p0:
                ps_w0 = [p0.tile([P, W], F32, name=f"ps0_{m}")
                         for m in range(QH + 1)]
                for k in range(KD):
                    g, kk = k // KPG, k % KPG
                    xg = xt_gs[(0, g)]
                    st, sp = (k == 0), (k == KD - 1)
                    for m in range(QH):
                        nc.tensor.matmul(ps_w0[m][:],
                                         wq_g[g][:, kk, m * HD:(m + 1) * HD],
                                         xg[:, kk, :], start=st, stop=sp)
                    nc.tensor.matmul(ps_w0[QH][:], wk_g[g][:, kk, :],
                                     xg[:, kk, :], start=st, stop=sp)
                for m in range(QH):
                    rope(ps_w0[m], qrot[:, m, 0:W], 0)
                rope(ps_w0[QH], krot[:, 0:W], 0)

            # steady-state psum pools: exactly 8 banks, with QKV/scores and
            # v/wo sharing slots (the streams interleave on the PE anyway).
            with tc.tile_pool(name="pmixs", bufs=2, space="PSUM") as pmixs, \
                 tc.tile_pool(name="pmixv", bufs=2, space="PSUM") as pmixv, \
                 tc.tile_pool(name="pso", bufs=2, space="PSUM") as pso, \
                 tc.tile_pool(name="pssum", bufs=2, space="PSUM") as pss, \
                 tc.tile_pool(name="ptp", bufs=3) as ptp, \
                 tc.tile_pool(name="attp", bufs=2) as attp:

                def emit_v(w):
                    for mm in range(W // P):
                        psv = pmixv.tile([P, OUTC], F32, tag="vw")
                        for k in range(KD):
                            g, kk = k // KPG, k % KPG
                            nc.tensor.matmul(
                                psv[:, 0:HD],
                                xt_gs[(w, g)][:, kk, mm * P:(mm + 1) * P],
                                wv_g[g][:, kk, :],
                                start=(k == 0), stop=(k == KD - 1))
                        nc.scalar.copy(v_sb[:, w * (W // P) + mm, :],
                                       psv[:, 0:HD])

                def emit_qkv(w):
                    s0 = w * W
                    if w + 1 < NW:
                        for g in range(KG):
                            load_xt_group(w + 1, g)
                    for m in range(QH + 1):
                        ps = pmixs.tile([P, W], F32, tag="s")
                        for k in range(KD):
                            g, kk = k // KPG, k % KPG
                            lhsT = (wq_g[g][:, kk, m * HD:(m + 1) * HD]
                                    if m < QH else wk_g[g][:, kk, :])
                            nc.tensor.matmul(ps[:], lhsT,
                                             xt_gs[(w, g)][:, kk, :],
                                             start=(k == 0), stop=(k == KD - 1))
                        dst = (qrot[:, m, s0:s0 + W] if m < QH
                               else krot[:, s0:s0 + W])
                        rope(ps, dst, s0)
                    emit_v(w)

                def emit_attention(qc):
                    q0 = qc * W
                    nkv = (W // P) * (qc + 1)
                    for h in range(QH):
                        ps_o = pso.tile([P, W], F32, tag="o")
                        ps_sum = pss.tile([P, W], F32, tag="sum")
                        for jj in range(nkv):
                            j0 = jj * P
                            ps_s = pmixs.tile([P, W], F32, tag="s")
                            nc.tensor.matmul(ps_s[:], krot[:, j0:j0 + P],
                                             qrot[:, h, q0:q0 + W],
                                             start=True, stop=True)
                            pt = ptp.tile([P, W], BF16, tag="pt")
                            nc.scalar.activation(pt[:], ps_s[:], Exp, scale=SCALE)
                            if jj >= (W // P) * qc:
                                nc.vector.tensor_mul(
                                    pt[:], pt[:],
                                    masks_sb[:, jj - (W // P) * qc, :])
                            nc.tensor.matmul(ps_o[:], v_sb[:, jj, :], pt[:],
                                             start=(jj == 0), stop=(jj == nkv - 1))
                            nc.tensor.matmul(ps_sum[:], ones_sb[:], pt[:],
                                             start=(jj == 0), stop=(jj == nkv - 1))
                        # normalization: reciprocal of one row, PE-broadcast to
                        # 128 partitions, multiply into the psum eviction.
                        inv1 = attp.tile([1, W], F32, tag="inv1")
                        nc.vector.reciprocal(inv1[:], ps_sum[0:1, :])
                        ps_bc = pmixs.tile([P, W], F32, tag="s")
                        nc.tensor.matmul(ps_bc[:], ones1f_sb[:], inv1[:],
                                         start=True, stop=True)
                        inv_bc = attp.tile([P, W], F32, tag="invbc")
                        nc.vector.tensor_copy(inv_bc[:], ps_bc[:])
                        at = attp.tile([P, W], BF16, tag="at")
                        nc.vector.tensor_mul(at[:], ps_o[:], inv_bc[:])
                        nc.sync.dma_start(out=ag_in[qc][h * HD:(h + 1) * HD, :],
                                          in_=at[:])
                    nc.gpsimd.collective_compute(
                        "AllGather", mybir.AluOpType.bypass,
                        replica_groups=[list(range(N_CORES))],
                        ins=[ag_in[qc][:]], outs=[ag_out[qc][:]])

                atfs: dict[int, bass.AP] = {}

                def load_atf(qc):
                    atf = atfp.tile([P, KD, W], BF16, tag="atf")
                    nc.sync.dma_start(
                        out=atf[:],
                        in_=ag_out[qc][:].rearrange("(k p) n -> p k n", p=P))
                    atfs[qc] = atf

                def emit_wo(qc):
                    q0 = qc * W
                    atf = atfs.pop(qc)
                    for mm in range(W // P):
                        m0 = mm * P
                        ps = pmixv.tile([P, OUTC], F32, tag="vw")
                        for k in range(KD):
                            nc.tensor.matmul(ps[:],
                                             atf[:, k, m0:m0 + P],
                                             wo_sb[:, k, :],
                                             start=(k == 0), stop=(k == KD - 1))
                        ot = outp.tile([P, OUTC], F32, tag="ot")
                        nc.vector.tensor_copy(ot[:], ps[:])
                        nc.sync.dma_start(out=out_d[q0 + m0:q0 + m0 + P, :],
                                          in_=ot[:])

                emit_v(0)
                emit_attention(0)
                emit_qkv(1)
                emit_attention(1)
                emit_qkv(2)
                emit_qkv(3)
                region_a.close()   # frees the weight/x^T pools for atf staging
                region_b = ExitStack()
                atfp = region_b.enter_context(tc.tile_pool(name="atfp", bufs=2, side="right"))
                outp = region_b.enter_context(tc.tile_pool(name="outp", bufs=2, side="right"))
                load_atf(0)
                load_atf(1)
                emit_attention(3)
                emit_wo(0)
                emit_attention(2)
                load_atf(3)
                emit_wo(1)
                load_atf(2)
                emit_wo(3)
                emit_wo(2)
                region_b.close()

    nc.compile()
    return nc


_NC_CACHE = None


def _get_nc():
    global _NC_CACHE
    if _NC_CACHE is None:
        _NC_CACHE = _build_nc()
    return _NC_CACHE


def make_in_maps(x, freqs_cos, freqs_sin, wq, wk, wv, wo):
    bf16 = ml_dtypes.bfloat16
    # half-split permutation: evens then odds within each head's 128 dims
    pidx = np.concatenate([np.arange(0, HD, 2), np.arange(1, HD, 2)])
    xt = np.ascontiguousarray(np.asarray(x).T.astype(bf16))
    cs = np.ascontiguousarray(np.asarray(freqs_cos).T.astype(np.float32))
    sn = np.ascontiguousarray(np.asarray(freqs_sin).T.astype(np.float32))
    wq = np.asarray(wq)
    wk = np.asarray(wk)
    wv = np.asarray(wv)
    wo = np.asarray(wo)
    in_maps = []
    for core in range(N_CORES):
        q_cols = np.concatenate([h * HD + pidx
                                 for h in range(QH * core, QH * (core + 1))])
        in_maps.append({
            "xt": xt,
            "wq": np.ascontiguousarray(wq[:, q_cols].astype(bf16)),
            "wk": np.ascontiguousarray(wk[:, core * HD + pidx].astype(bf16)),
            "wv": np.ascontiguousarray(wv[:, core * HD:(core + 1) * HD].astype(bf16)),
            "wo": np.ascontiguousarray(wo[:, core * OUTC:(core + 1) * OUTC].astype(bf16)),
            "cs": cs,
            "sn": sn,
        })
    return in_maps


def kernel(x, freqs_cos, freqs_sin, wq, wk, wv, wo, _run_kwargs=None):
    in_maps = make_in_maps(x, freqs_cos, freqs_sin, wq, wk, wv, wo)
    nc = _get_nc()
    res = run_bass_kernel_spmd(nc, in_maps, list(range(N_CORES)),
                               **(_run_kwargs or {}))
    out = np.concatenate([res.results[i]["out"] for i in range(N_CORES)], axis=1)
    if _run_kwargs is not None:
        kernel.last_results = res
    return np.ascontiguousarray(out.astype(np.float32))


# revision 16
# speedup vs baseline: 1.2380x; 1.0699x over previous
"""Trainium2 Bass kernel for a GQA causal-attention block (TP over heads, 8 cores).

Computation (per reference): q/k/v projections of x, interleaved RoPE on q/k,
GQA causal attention (32 q heads, 8 kv heads, head_dim 128, seq 2048), output
projection. Sharding: tensor-parallel over heads — each core owns 4 q heads and
their shared kv head. The attention output (transposed layout) is AllGathered
across cores per 512-sequence window and each core computes a 512-column slice
of the final output projection; the host concatenates the column slices.

Device dataflow notes:
  - Everything transposed: x^T streams as the matmul moving operand so q^T/k^T
    come out with head_dim on partitions; scores are computed transposed
    (s^T[k_pos, q_pos]) so exp(s^T) feeds the PV matmul directly as the moving
    operand without any on-chip transposes.
  - RoPE uses a half-split head_dim permutation (evens then odds), folded into
    the wq/wk columns on the host, so the rotation is two 64-partition
    multiply/add pairs against host-transposed cos/sin tables.
  - Softmax skips the max subtraction (scores ~ N(0,1) after scaling); row sums
    come from a ones-matmul accumulated alongside PV; normalization is folded
    into the psum->sbuf eviction of the attention output via a PE-broadcast
    reciprocal row.
  - Causal masking multiplies exp(scores) by a 0/1 mask on diagonal blocks only.
  - Global software pipeline keeps the PE stream dense (no HAM re-throttle):
    QKV(0) runs contraction-outer behind the initial DMAs, then attention
    window qc is emitted right after QKV(qc) (causality makes window qc fully
    computable then), AllGathers fire per window, and each window's output
    projection is emitted several phases later so collective + gather-DMA
    latency hides under guaranteed PE work. Attention windows run in order
    0,1,3,2 so the final output-projection slice has cover for its AllGather.
  - PSUM is exactly 8 banks: window-0 k-outer uses 5 transient banks; steady
    state shares 2 banks between QKV-psum/scores, 2 between v/wo-psum, plus
    2 PV accumulators and 2 softmax-sum accumulators.
"""

import numpy as np
import ml_dtypes

import concourse.bass as bass
import concourse.mybir as mybir
import concourse.tile as tile
from concourse import bacc
from concourse.bass_utils import run_bass_kernel_spmd

N_CORES = 8
P = 128
SEQ = 2048
DIM = 4096
N_HEADS = 32
N_KV_HEADS = 8
HD = 128
QH = N_HEADS // N_CORES        # q heads per core
KD = DIM // P                  # contraction chunks
KG = 4                         # k-chunk DMA groups
KPG = KD // KG                 # k chunks per group
W = 512                        # seq window (matmul moving free dim)
NW = SEQ // W
NT = SEQ // P
OUTC = DIM // N_CORES          # output columns per core
SCALE = HD ** -0.5

BF16 = mybir.dt.bfloat16
F32 = mybir.dt.float32


def _build_nc():
    nc = bacc.Bacc("TRN2", target_bir_lowering=False, debug=False,
                   num_devices=N_CORES)

    xt_d = nc.dram_tensor("xt", [DIM, SEQ], BF16, kind="ExternalInput")
    wq_d = nc.dram_tensor("wq", [DIM, QH * HD], BF16, kind="ExternalInput")
    wk_d = nc.dram_tensor("wk", [DIM, HD], BF16, kind="ExternalInput")
    wv_d = nc.dram_tensor("wv", [DIM, HD], BF16, kind="ExternalInput")
    wo_d = nc.dram_tensor("wo", [DIM, OUTC], BF16, kind="ExternalInput")
    cs_d = nc.dram_tensor("cs", [64, SEQ], F32, kind="ExternalInput")
    sn_d = nc.dram_tensor("sn", [64, SEQ], F32, kind="ExternalInput")
    out_d = nc.dram_tensor("out", [SEQ, OUTC], F32, kind="ExternalOutput")

    # 0/1 causal masks for the 4 diagonal alignments of a [128 kv, 512 q] block:
    # mask[p, c, q] = 1 iff kv offset p + c*128 <= q (within the 512-q window).
    j = np.arange(P)[:, None, None]
    c = np.arange(4)[None, :, None]
    q = np.arange(W)[None, None, :]
    masks_np = (j + c * P <= q).astype(ml_dtypes.bfloat16)
    masks_d = nc.inline_tensor(masks_np, "cmasks")
    ones_d = nc.inline_tensor(np.ones([P, P], dtype=ml_dtypes.bfloat16), "ones")
    ones1f_d = nc.inline_tensor(np.ones([1, P], dtype=np.float32), "ones1f")

    Exp = mybir.ActivationFunctionType.Exp

    from contextlib import ExitStack
    with tile.TileContext(nc) as tc:
        with tc.tile_pool(name="const", bufs=1) as constp, \
             tc.tile_pool(name="acts", bufs=1) as actp, \
             tc.tile_pool(name="ropet", bufs=1) as rtp, \
             tc.tile_pool(name="ccdram", bufs=1, space="DRAM") as dcc, \
             tc.tile_pool(name="wop", bufs=1) as wop:
            region_a = ExitStack()
            wp = region_a.enter_context(tc.tile_pool(name="wqkv", bufs=1, side="right"))
            xp = region_a.enter_context(tc.tile_pool(name="xtp", bufs=2, side="right"))
            qrot = actp.tile([P, QH, SEQ], BF16)   # q^T (rope'd), per head
            krot = actp.tile([P, SEQ], BF16)       # k^T (rope'd)
            v_sb = actp.tile([P, NT, HD], BF16)    # v row-tiles [seq, hd]

            wq_g = [wp.tile([P, KPG, QH * HD], BF16, name=f"wq{g}")
                    for g in range(KG)]
            wk_g = [wp.tile([P, KPG, HD], BF16, name=f"wk{g}")
                    for g in range(KG)]
            wv_g = [wp.tile([P, KPG, HD], BF16, name=f"wv{g}")
                    for g in range(KG)]
            xt_gs: dict[tuple[int, int], bass.AP] = {}

            def load_xt_group(w, g):
                t = xp.tile([P, KPG, W], BF16, tag=f"xt{g}")
                k0 = g * KPG
                nc.sync.dma_start(
                    out=t[:],
                    in_=xt_d[k0 * P:(k0 + KPG) * P, w * W:(w + 1) * W]
                    .rearrange("(k p) n -> p k n", p=P))
                xt_gs[(w, g)] = t

            def grp(handle, g):
                return handle[g * KPG * P:(g + 1) * KPG * P, :] \
                    .rearrange("(k p) n -> p k n", p=P)

            # initial DMA order: window-0 groups interleaved with weights, then
            # window 1, with the small tables slotted where they're needed.
            for g in range(KG):
                load_xt_group(0, g)
                nc.scalar.dma_start(out=wq_g[g][:], in_=grp(wq_d, g))
                nc.scalar.dma_start(out=wk_g[g][:], in_=grp(wk_d, g))
                nc.scalar.dma_start(out=wv_g[g][:], in_=grp(wv_d, g))
            load_xt_group(1, 0)
            load_xt_group(1, 1)
            trig_sb = constp.tile([P, SEQ], F32)
            nc.scalar.dma_start(out=trig_sb[0:64, :], in_=cs_d[:])
            nc.scalar.dma_start(out=trig_sb[64:128, :], in_=sn_d[:])
            load_xt_group(1, 2)
            load_xt_group(1, 3)
            masks_sb = constp.tile([P, 4, W], BF16)
            nc.scalar.dma_start(out=masks_sb[:], in_=masks_d[:])
            ones_sb = constp.tile([P, P], BF16)
            nc.scalar.dma_start(out=ones_sb[:], in_=ones_d[:])
            ones1f_sb = constp.tile([1, P], F32)
            nc.scalar.dma_start(out=ones1f_sb[:], in_=ones1f_d[:])
            wo_sb = wop.tile([P, KD, OUTC], BF16)

            ag_in = [dcc.tile([QH * HD, W], BF16, name=f"agin{w}")
                     for w in range(NW)]
            ag_out = [dcc.tile([DIM, W], BF16, addr_space="Shared",
                               name=f"agout{w}")
                      for w in range(NW)]

            def rope(ps, dst, s0):
                cw = trig_sb[0:64, s0:s0 + W]
                sw = trig_sb[64:128, s0:s0 + W]
                t1 = rtp.tile([64, W], F32, tag="t1")
                t2 = rtp.tile([64, W], F32, tag="t2")
                # even half: x1*cos - x2*sin
                nc.vector.tensor_mul(t1[:], ps[0:64, :], cw)
                nc.vector.tensor_mul(t2[:], ps[64:128, :], sw)
                nc.vector.tensor_sub(dst[0:64], t1[:], t2[:])
                # odd half: x1*sin + x2*cos (reuse the t1/t2 slots)
                t3 = rtp.tile([64, W], F32, tag="t1")
                t4 = rtp.tile([64, W], F32, tag="t2")
                nc.vector.tensor_mul(t3[:], ps[0:64, :], sw)
                nc.vector.tensor_mul(t4[:], ps[64:128, :], cw)
                nc.vector.tensor_add(dst[64:128], t3[:], t4[:])

            # window 0 q/k: contraction-outer so PE streams behind the DMAs.
            # Each psum tile owns a full bank (start=True zeroes a whole 2KB
            # zero-region, so accumulation groups must never share a bank).
            with tc.tile_pool(name="ps0", bufs=1, space="PSUM") as p0:
                ps_w0 = [p0.tile([P, W], F32, name=f"ps0_{m}")
                         for m in range(QH + 1)]
                for k in range(KD):
                    g, kk = k // KPG, k % KPG
                    xg = xt_gs[(0, g)]
                    st, sp = (k == 0), (k == KD - 1)
                    for m in range(QH):
                        nc.tensor.matmul(ps_w0[m][:],
                                         wq_g[g][:, kk, m * HD:(m + 1) * HD],
                                         xg[:, kk, :], start=st, stop=sp)
                    nc.tensor.matmul(ps_w0[QH][:], wk_g[g][:, kk, :],
                                     xg[:, kk, :], start=st, stop=sp)
                for m in range(QH):
                    rope(ps_w0[m], qrot[:, m, 0:W], 0)
                rope(ps_w0[QH], krot[:, 0:W], 0)

            # steady-state psum pools: exactly 8 banks, with QKV/scores and
            # v/wo sharing slots (the streams interleave on the PE anyway).
            with tc.tile_pool(name="pmixs", bufs=2, space="PSUM") as pmixs, \
                 tc.tile_pool(name="pmixv", bufs=2, space="PSUM") as pmixv, \
                 tc.tile_pool(name="pso", bufs=2, space="PSUM") as pso, \
                 tc.tile_pool(name="pssum", bufs=2, space="PSUM") as pss, \
                 tc.tile_pool(name="ptp", bufs=3) as ptp, \
                 tc.tile_pool(name="attp", bufs=2) as attp:

                def emit_v(w):
                    for mm in range(W // P):
                        psv = pmixv.tile([P, OUTC], F32, tag="vw")
                        for k in range(KD):
                            g, kk = k // KPG, k % KPG
                            nc.tensor.matmul(
                                psv[:, 0:HD],
                                xt_gs[(w, g)][:, kk, mm * P:(mm + 1) * P],
                                wv_g[g][:, kk, :],
                                start=(k == 0), stop=(k == KD - 1))
                        nc.scalar.copy(v_sb[:, w * (W // P) + mm, :],
                                       psv[:, 0:HD])

                def emit_qkv(w):
                    s0 = w * W
                    if w + 1 < NW:
                        for g in range(KG):
                            load_xt_group(w + 1, g)
                    for m in range(QH + 1):
                        ps = pmixs.tile([P, W], F32, tag="s")
                        for k in range(KD):
                            g, kk = k // KPG, k % KPG
                            lhsT = (wq_g[g][:, kk, m * HD:(m + 1) * HD]
                                    if m < QH else wk_g[g][:, kk, :])
                            nc.tensor.matmul(ps[:], lhsT,
                                             xt_gs[(w, g)][:, kk, :],
                                             start=(k == 0), stop=(k == KD - 1))
                        dst = (qrot[:, m, s0:s0 + W] if m < QH
                               else krot[:, s0:s0 + W])
                        rope(ps, dst, s0)
                    emit_v(w)

                def emit_attention(qc):
                    q0 = qc * W
                    nkv = (W // P) * (qc + 1)

                    def finish(ps_o, ps_sum, h):
                        # normalization: reciprocal of one row, PE-broadcast to
                        # 128 partitions, multiply into the psum eviction. The
                        # reciprocal was issued a head earlier, so the PE never
                        # waits on it.
                        ps_bc = pmixs.tile([P, W], F32, tag="s")
                        nc.tensor.matmul(ps_bc[:], ones1f_sb[:], invs[h][:],
                                         start=True, stop=True)
                        inv_bc = attp.tile([P, W], F32, tag="invbc")
                        nc.vector.tensor_copy(inv_bc[:], ps_bc[:])
                        at = attp.tile([P, W], BF16, tag="at")
                        nc.vector.tensor_mul(at[:], ps_o[:], inv_bc[:])
                        nc.sync.dma_start(out=ag_in[qc][h * HD:(h + 1) * HD, :],
                                          in_=at[:])

                    invs = {}
                    pend = None
                    for h in range(QH):
                        ps_o = pso.tile([P, W], F32, tag="o")
                        ps_sum = pss.tile([P, W], F32, tag="sum")
                        for jj in range(nkv):
                            j0 = jj * P
                            ps_s = pmixs.tile([P, W], F32, tag="s")
                            nc.tensor.matmul(ps_s[:], krot[:, j0:j0 + P],
                                             qrot[:, h, q0:q0 + W],
                                             start=True, stop=True)
                            pt = ptp.tile([P, W], BF16, tag="pt")
                            nc.scalar.activation(pt[:], ps_s[:], Exp, scale=SCALE)
                            if jj >= (W // P) * qc:
                                nc.vector.tensor_mul(
                                    pt[:], pt[:],
                                    masks_sb[:, jj - (W // P) * qc, :])
                            nc.tensor.matmul(ps_o[:], v_sb[:, jj, :], pt[:],
                                             start=(jj == 0), stop=(jj == nkv - 1))
                            nc.tensor.matmul(ps_sum[:], ones_sb[:], pt[:],
                                             start=(jj == 0), stop=(jj == nkv - 1))
                        inv1 = attp.tile([1, W], F32, tag="inv1")
                        nc.vector.reciprocal(inv1[:], ps_sum[0:1, :])
                        invs[h] = inv1
                        if pend is not None:
                            finish(*pend)
                        pend = (ps_o, ps_sum, h)
                    finish(*pend)
                    nc.gpsimd.collective_compute(
                        "AllGather", mybir.AluOpType.bypass,
                        replica_groups=[list(range(N_CORES))],
                        ins=[ag_in[qc][:]], outs=[ag_out[qc][:]])

                atfs: dict[int, bass.AP] = {}

                def load_atf(qc):
                    atf = atfp.tile([P, KD, W], BF16, tag="atf")
                    nc.sync.dma_start(
                        out=atf[:],
                        in_=ag_out[qc][:].rearrange("(k p) n -> p k n", p=P))
                    atfs[qc] = atf

                def emit_wo(qc):
                    q0 = qc * W
                    atf = atfs.pop(qc)
                    for mm in range(W // P):
                        m0 = mm * P
                        ps = pmixv.tile([P, OUTC], F32, tag="vw")
                        for k in range(KD):
                            nc.tensor.matmul(ps[:],
                                             atf[:, k, m0:m0 + P],
                                             wo_sb[:, k, :],
                                             start=(k == 0), stop=(k == KD - 1))
                        ot = outp.tile([P, OUTC], F32, tag="ot")
                        nc.vector.tensor_copy(ot[:], ps[:])
                        nc.sync.dma_start(out=out_d[q0 + m0:q0 + m0 + P, :],
                                          in_=ot[:])

                emit_v(0)
                emit_attention(0)
                emit_qkv(1)
                emit_attention(1)
                emit_qkv(2)
                nc.scalar.dma_start(out=wo_sb[:],
                                    in_=wo_d[:].rearrange("(k p) n -> p k n", p=P))
                emit_qkv(3)
                region_a.close()   # frees the weight/x^T pools for atf staging
                region_b = ExitStack()
                atfp = region_b.enter_context(tc.tile_pool(name="atfp", bufs=2, side="right"))
                outp = region_b.enter_context(tc.tile_pool(name="outp", bufs=2, side="right"))
                load_atf(0)
                load_atf(1)
                emit_attention(3)
                emit_wo(0)
                emit_attention(2)
                load_atf(3)
                emit_wo(1)
                load_atf(2)
                emit_wo(3)
                emit_wo(2)
                region_b.close()

    nc.compile()
    return nc


_NC_CACHE = None


def _get_nc():
    global _NC_CACHE
    if _NC_CACHE is None:
        _NC_CACHE = _build_nc()
    return _NC_CACHE


def make_in_maps(x, freqs_cos, freqs_sin, wq, wk, wv, wo):
    bf16 = ml_dtypes.bfloat16
    # half-split permutation: evens then odds within each head's 128 dims
    pidx = np.concatenate([np.arange(0, HD, 2), np.arange(1, HD, 2)])
    xt = np.ascontiguousarray(np.asarray(x).T.astype(bf16))
    cs = np.ascontiguousarray(np.asarray(freqs_cos).T.astype(np.float32))
    sn = np.ascontiguousarray(np.asarray(freqs_sin).T.astype(np.float32))
    wq = np.asarray(wq)
    wk = np.asarray(wk)
    wv = np.asarray(wv)
    wo = np.asarray(wo)
    in_maps = []
    for core in range(N_CORES):
        q_cols = np.concatenate([h * HD + pidx
                                 for h in range(QH * core, QH * (core + 1))])
        in_maps.append({
            "xt": xt,
            "wq": np.ascontiguousarray(wq[:, q_cols].astype(bf16)),
            "wk": np.ascontiguousarray(wk[:, core * HD + pidx].astype(bf16)),
            "wv": np.ascontiguousarray(wv[:, core * HD:(core + 1) * HD].astype(bf16)),
            "wo": np.ascontiguousarray(wo[:, core * OUTC:(core + 1) * OUTC].astype(bf16)),
            "cs": cs,
            "sn": sn,
        })
    return in_maps


def kernel(x, freqs_cos, freqs_sin, wq, wk, wv, wo, _run_kwargs=None):
    in_maps = make_in_maps(x, freqs_cos, freqs_sin, wq, wk, wv, wo)
    nc = _get_nc()
    res = run_bass_kernel_spmd(nc, in_maps, list(range(N_CORES)),
                               **(_run_kwargs or {}))
    out = np.concatenate([res.results[i]["out"] for i in range(N_CORES)], axis=1)
    if _run_kwargs is not None:
        kernel.last_results = res
    return np.ascontiguousarray(out.astype(np.float32))


# revision 17
# speedup vs baseline: 1.2587x; 1.0167x over previous
"""Trainium2 Bass kernel for a GQA causal-attention block (TP over heads, 8 cores).

Computation (per reference): q/k/v projections of x, interleaved RoPE on q/k,
GQA causal attention (32 q heads, 8 kv heads, head_dim 128, seq 2048), output
projection. Sharding: tensor-parallel over heads — each core owns 4 q heads and
their shared kv head. The attention output (transposed layout) is AllGathered
across cores per 512-sequence window and each core computes a 512-column slice
of the final output projection; the host concatenates the column slices.

Device dataflow notes:
  - Everything transposed: x^T streams as the matmul moving operand so q^T/k^T
    come out with head_dim on partitions; scores are computed transposed
    (s^T[k_pos, q_pos]) so exp(s^T) feeds the PV matmul directly as the moving
    operand without any on-chip transposes.
  - RoPE uses a half-split head_dim permutation (evens then odds), folded into
    the wq/wk columns on the host, so the rotation is two 64-partition
    multiply/add pairs against host-transposed cos/sin tables.
  - Softmax skips the max subtraction (scores ~ N(0,1) after scaling); row sums
    come from a ones-matmul accumulated alongside PV; normalization is folded
    into the psum->sbuf eviction of the attention output via a PE-broadcast
    reciprocal row.
  - Causal masking multiplies exp(scores) by a 0/1 mask on diagonal blocks only.
  - Global software pipeline keeps the PE stream dense (no HAM re-throttle):
    QKV(0) runs contraction-outer behind the initial DMAs, then attention
    window qc is emitted right after QKV(qc) (causality makes window qc fully
    computable then), AllGathers fire per window, and each window's output
    projection is emitted several phases later so collective + gather-DMA
    latency hides under guaranteed PE work. Attention windows run in order
    0,1,3,2 so the final output-projection slice has cover for its AllGather.
  - PSUM is exactly 8 banks: window-0 k-outer uses 5 transient banks; steady
    state shares 2 banks between QKV-psum/scores, 2 between v/wo-psum, plus
    2 PV accumulators and 2 softmax-sum accumulators.
"""

import numpy as np
import ml_dtypes

import concourse.bass as bass
import concourse.mybir as mybir
import concourse.tile as tile
from concourse import bacc
from concourse.bass_utils import run_bass_kernel_spmd

N_CORES = 8
P = 128
SEQ = 2048
DIM = 4096
N_HEADS = 32
N_KV_HEADS = 8
HD = 128
QH = N_HEADS // N_CORES        # q heads per core
KD = DIM // P                  # contraction chunks
KG = 4                         # k-chunk DMA groups
KPG = KD // KG                 # k chunks per group
W = 512                        # seq window (matmul moving free dim)
NW = SEQ // W
NT = SEQ // P
OUTC = DIM // N_CORES          # output columns per core
SCALE = HD ** -0.5

BF16 = mybir.dt.bfloat16
F32 = mybir.dt.float32


def _build_nc():
    nc = bacc.Bacc("TRN2", target_bir_lowering=False, debug=False,
                   num_devices=N_CORES)

    xt_d = nc.dram_tensor("xt", [NW, P, KD, W], BF16, kind="ExternalInput")
    wq_d = nc.dram_tensor("wq", [P, KD, QH * HD], BF16, kind="ExternalInput")
    wk_d = nc.dram_tensor("wk", [P, KD, HD], BF16, kind="ExternalInput")
    wv_d = nc.dram_tensor("wv", [P, KD, HD], BF16, kind="ExternalInput")
    wo_d = nc.dram_tensor("wo", [P, KD, OUTC], BF16, kind="ExternalInput")
    cs_d = nc.dram_tensor("cs", [64, SEQ], F32, kind="ExternalInput")
    sn_d = nc.dram_tensor("sn", [64, SEQ], F32, kind="ExternalInput")
    out_d = nc.dram_tensor("out", [SEQ, OUTC], F32, kind="ExternalOutput")

    # 0/1 causal masks for the 4 diagonal alignments of a [128 kv, 512 q] block:
    # mask[p, c, q] = 1 iff kv offset p + c*128 <= q (within the 512-q window).
    j = np.arange(P)[:, None, None]
    c = np.arange(4)[None, :, None]
    q = np.arange(W)[None, None, :]
    masks_np = (j + c * P <= q).astype(ml_dtypes.bfloat16)
    masks_d = nc.inline_tensor(masks_np, "cmasks")
    ones_d = nc.inline_tensor(np.ones([P, P], dtype=ml_dtypes.bfloat16), "ones")
    ones1f_d = nc.inline_tensor(np.ones([1, P], dtype=np.float32), "ones1f")

    Exp = mybir.ActivationFunctionType.Exp

    from contextlib import ExitStack
    with tile.TileContext(nc) as tc:
        with tc.tile_pool(name="const", bufs=1) as constp, \
             tc.tile_pool(name="acts", bufs=1) as actp, \
             tc.tile_pool(name="ropet", bufs=1) as rtp, \
             tc.tile_pool(name="ccdram", bufs=1, space="DRAM") as dcc, \
             tc.tile_pool(name="wop", bufs=1) as wop:
            region_a = ExitStack()
            wp = region_a.enter_context(tc.tile_pool(name="wqkv", bufs=1, side="right"))
            xp = region_a.enter_context(tc.tile_pool(name="xtp", bufs=2, side="right"))
            qrot = actp.tile([P, QH, SEQ], BF16)   # q^T (rope'd), per head
            krot = actp.tile([P, SEQ], BF16)       # k^T (rope'd)
            v_sb = actp.tile([P, NT, HD], BF16)    # v row-tiles [seq, hd]

            wq_g = [wp.tile([P, KPG, QH * HD], BF16, name=f"wq{g}")
                    for g in range(KG)]
            wk_g = [wp.tile([P, KPG, HD], BF16, name=f"wk{g}")
                    for g in range(KG)]
            wv_g = [wp.tile([P, KPG, HD], BF16, name=f"wv{g}")
                    for g in range(KG)]
            xt_gs: dict[tuple[int, int], bass.AP] = {}

            def load_xt_group(w, g):
                t = xp.tile([P, KPG, W], BF16, tag=f"xt{g}")
                nc.sync.dma_start(
                    out=t[:], in_=xt_d[w, :, g * KPG:(g + 1) * KPG, :])
                xt_gs[(w, g)] = t

            def grp(handle, g):
                return handle[:, g * KPG:(g + 1) * KPG, :]

            # initial DMA order: window-0 groups interleaved with weights, then
            # window 1, with the small tables slotted where they're needed.
            for g in range(KG):
                load_xt_group(0, g)
                nc.scalar.dma_start(out=wq_g[g][:], in_=grp(wq_d, g))
                nc.scalar.dma_start(out=wk_g[g][:], in_=grp(wk_d, g))
            load_xt_group(1, 0)
            load_xt_group(1, 1)
            trig_sb = constp.tile([P, SEQ], F32)
            nc.scalar.dma_start(out=trig_sb[0:64, :], in_=cs_d[:])
            nc.scalar.dma_start(out=trig_sb[64:128, :], in_=sn_d[:])
            load_xt_group(1, 2)
            load_xt_group(1, 3)
            for g in range(KG):
                nc.scalar.dma_start(out=wv_g[g][:], in_=grp(wv_d, g))
            masks_sb = constp.tile([P, 4, W], BF16)
            nc.scalar.dma_start(out=masks_sb[:], in_=masks_d[:])
            ones_sb = constp.tile([P, P], BF16)
            nc.scalar.dma_start(out=ones_sb[:], in_=ones_d[:])
            ones1f_sb = constp.tile([1, P], F32)
            nc.scalar.dma_start(out=ones1f_sb[:], in_=ones1f_d[:])
            wo_sb = wop.tile([P, KD, OUTC], BF16)

            ag_in = [dcc.tile([QH * HD, W], BF16, name=f"agin{w}")
                     for w in range(NW)]
            ag_out = [dcc.tile([DIM, W], BF16, addr_space="Shared",
                               name=f"agout{w}")
                      for w in range(NW)]

            def rope(ps, dst, s0):
                cw = trig_sb[0:64, s0:s0 + W]
                sw = trig_sb[64:128, s0:s0 + W]
                t1 = rtp.tile([64, W], F32, tag="t1")
                t2 = rtp.tile([64, W], F32, tag="t2")
                # even half: x1*cos - x2*sin
                nc.vector.tensor_mul(t1[:], ps[0:64, :], cw)
                nc.vector.tensor_mul(t2[:], ps[64:128, :], sw)
                nc.vector.tensor_sub(dst[0:64], t1[:], t2[:])
                # odd half: x1*sin + x2*cos (reuse the t1/t2 slots)
                t3 = rtp.tile([64, W], F32, tag="t1")
                t4 = rtp.tile([64, W], F32, tag="t2")
                nc.vector.tensor_mul(t3[:], ps[0:64, :], sw)
                nc.vector.tensor_mul(t4[:], ps[64:128, :], cw)
                nc.vector.tensor_add(dst[64:128], t3[:], t4[:])

            # window 0 q/k: contraction-outer so PE streams behind the DMAs.
            # Each psum tile owns a full bank (start=True zeroes a whole 2KB
            # zero-region, so accumulation groups must never share a bank).
            with tc.tile_pool(name="ps0", bufs=1, space="PSUM") as p0:
                ps_w0 = [p0.tile([P, W], F32, name=f"ps0_{m}")
                         for m in range(QH + 1)]
                for k in range(KD):
                    g, kk = k // KPG, k % KPG
                    xg = xt_gs[(0, g)]
                    st, sp = (k == 0), (k == KD - 1)
                    for m in range(QH):
                        nc.tensor.matmul(ps_w0[m][:],
                                         wq_g[g][:, kk, m * HD:(m + 1) * HD],
                                         xg[:, kk, :], start=st, stop=sp)
                    nc.tensor.matmul(ps_w0[QH][:], wk_g[g][:, kk, :],
                                     xg[:, kk, :], start=st, stop=sp)
                for m in range(QH):
                    rope(ps_w0[m], qrot[:, m, 0:W], 0)
                rope(ps_w0[QH], krot[:, 0:W], 0)

            # steady-state psum pools: exactly 8 banks, with QKV/scores and
            # v/wo sharing slots (the streams interleave on the PE anyway).
            with tc.tile_pool(name="pmixs", bufs=2, space="PSUM") as pmixs, \
                 tc.tile_pool(name="pmixv", bufs=2, space="PSUM") as pmixv, \
                 tc.tile_pool(name="pso", bufs=2, space="PSUM") as pso, \
                 tc.tile_pool(name="pssum", bufs=2, space="PSUM") as pss, \
                 tc.tile_pool(name="ptp", bufs=3) as ptp, \
                 tc.tile_pool(name="attp", bufs=2) as attp:

                def emit_v(w):
                    for mm in range(W // P):
                        psv = pmixv.tile([P, OUTC], F32, tag="vw")
                        for k in range(KD):
                            g, kk = k // KPG, k % KPG
                            nc.tensor.matmul(
                                psv[:, 0:HD],
                                xt_gs[(w, g)][:, kk, mm * P:(mm + 1) * P],
                                wv_g[g][:, kk, :],
                                start=(k == 0), stop=(k == KD - 1))
                        nc.scalar.copy(v_sb[:, w * (W // P) + mm, :],
                                       psv[:, 0:HD])

                def emit_qkv(w):
                    s0 = w * W
                    if w + 1 < NW:
                        for g in range(KG):
                            load_xt_group(w + 1, g)
                    for m in range(QH + 1):
                        ps = pmixs.tile([P, W], F32, tag="s")
                        for k in range(KD):
                            g, kk = k // KPG, k % KPG
                            lhsT = (wq_g[g][:, kk, m * HD:(m + 1) * HD]
                                    if m < QH else wk_g[g][:, kk, :])
                            nc.tensor.matmul(ps[:], lhsT,
                                             xt_gs[(w, g)][:, kk, :],
                                             start=(k == 0), stop=(k == KD - 1))
                        dst = (qrot[:, m, s0:s0 + W] if m < QH
                               else krot[:, s0:s0 + W])
                        rope(ps, dst, s0)
                    emit_v(w)

                def emit_attention(qc):
                    q0 = qc * W
                    nkv = (W // P) * (qc + 1)

                    def finish(ps_o, ps_sum, h):
                        # normalization: reciprocal of one row, PE-broadcast to
                        # 128 partitions, multiply into the psum eviction. The
                        # reciprocal was issued a head earlier, so the PE never
                        # waits on it.
                        ps_bc = pmixs.tile([P, W], F32, tag="s")
                        nc.tensor.matmul(ps_bc[:], ones1f_sb[:], invs[h][:],
                                         start=True, stop=True)
                        inv_bc = attp.tile([P, W], F32, tag="invbc")
                        nc.vector.tensor_copy(inv_bc[:], ps_bc[:])
                        at = attp.tile([P, W], BF16, tag="at")
                        nc.vector.tensor_mul(at[:], ps_o[:], inv_bc[:])
                        nc.sync.dma_start(out=ag_in[qc][h * HD:(h + 1) * HD, :],
                                          in_=at[:])

                    invs = {}
                    pend = None
                    for h in range(QH):
                        ps_o = pso.tile([P, W], F32, tag="o")
                        ps_sum = pss.tile([P, W], F32, tag="sum")
                        for jj in range(nkv):
                            j0 = jj * P
                            ps_s = pmixs.tile([P, W], F32, tag="s")
                            nc.tensor.matmul(ps_s[:], krot[:, j0:j0 + P],
                                             qrot[:, h, q0:q0 + W],
                                             start=True, stop=True)
                            pt = ptp.tile([P, W], BF16, tag="pt")
                            nc.scalar.activation(pt[:], ps_s[:], Exp, scale=SCALE)
                            if jj >= (W // P) * qc:
                                nc.vector.tensor_mul(
                                    pt[:], pt[:],
                                    masks_sb[:, jj - (W // P) * qc, :])
                            nc.tensor.matmul(ps_o[:], v_sb[:, jj, :], pt[:],
                                             start=(jj == 0), stop=(jj == nkv - 1))
                            nc.tensor.matmul(ps_sum[:], ones_sb[:], pt[:],
                                             start=(jj == 0), stop=(jj == nkv - 1))
                        inv1 = attp.tile([1, W], F32, tag="inv1")
                        nc.vector.reciprocal(inv1[:], ps_sum[0:1, :])
                        invs[h] = inv1
                        if pend is not None:
                            finish(*pend)
                        pend = (ps_o, ps_sum, h)
                    finish(*pend)
                    nc.gpsimd.collective_compute(
                        "AllGather", mybir.AluOpType.bypass,
                        replica_groups=[list(range(N_CORES))],
                        ins=[ag_in[qc][:]], outs=[ag_out[qc][:]])

                atfs: dict[int, bass.AP] = {}

                def load_atf(qc):
                    atf = atfp.tile([P, KD, W], BF16, tag="atf")
                    nc.sync.dma_start(
                        out=atf[:],
                        in_=ag_out[qc][:].rearrange("(k p) n -> p k n", p=P))
                    atfs[qc] = atf

                def emit_wo(qc):
                    q0 = qc * W
                    atf = atfs.pop(qc)
                    for mm in range(W // P):
                        m0 = mm * P
                        ps = pmixv.tile([P, OUTC], F32, tag="vw")
                        for k in range(KD):
                            nc.tensor.matmul(ps[:],
                                             atf[:, k, m0:m0 + P],
                                             wo_sb[:, k, :],
                                             start=(k == 0), stop=(k == KD - 1))
                        ot = outp.tile([P, OUTC], F32, tag="ot")
                        nc.vector.tensor_copy(ot[:], ps[:])
                        nc.sync.dma_start(out=out_d[q0 + m0:q0 + m0 + P, :],
                                          in_=ot[:])

                emit_v(0)
                emit_attention(0)
                emit_qkv(1)
                emit_attention(1)
                emit_qkv(2)
                nc.scalar.dma_start(out=wo_sb[:], in_=wo_d[:])
                emit_qkv(3)
                region_a.close()   # frees the weight/x^T pools for atf staging
                region_b = ExitStack()
                atfp = region_b.enter_context(tc.tile_pool(name="atfp", bufs=3, side="right"))
                outp = region_b.enter_context(tc.tile_pool(name="outp", bufs=2, side="right"))
                load_atf(0)
                load_atf(1)
                emit_attention(3)
                emit_attention(2)
                load_atf(3)
                emit_wo(0)
                emit_wo(1)
                load_atf(2)
                emit_wo(3)
                emit_wo(2)
                region_b.close()

    nc.compile()
    return nc


_NC_CACHE = None


def _get_nc():
    global _NC_CACHE
    if _NC_CACHE is None:
        _NC_CACHE = _build_nc()
    return _NC_CACHE


def make_in_maps(x, freqs_cos, freqs_sin, wq, wk, wv, wo):
    bf16 = ml_dtypes.bfloat16
    # half-split permutation: evens then odds within each head's 128 dims
    pidx = np.concatenate([np.arange(0, HD, 2), np.arange(1, HD, 2)])

    def ptile(w):
        # [DIM, cols] -> [P, KD, cols] so per-partition DMA runs are contiguous
        return np.ascontiguousarray(
            w.reshape(KD, P, w.shape[1]).transpose(1, 0, 2).astype(bf16))

    xt = np.ascontiguousarray(
        np.asarray(x).T.astype(bf16).reshape(KD, P, NW, W).transpose(2, 1, 0, 3))
    cs = np.ascontiguousarray(np.asarray(freqs_cos).T.astype(np.float32))
    sn = np.ascontiguousarray(np.asarray(freqs_sin).T.astype(np.float32))
    wq = np.asarray(wq)
    wk = np.asarray(wk)
    wv = np.asarray(wv)
    wo = np.asarray(wo)
    in_maps = []
    for core in range(N_CORES):
        q_cols = np.concatenate([h * HD + pidx
                                 for h in range(QH * core, QH * (core + 1))])
        in_maps.append({
            "xt": xt,
            "wq": ptile(wq[:, q_cols]),
            "wk": ptile(wk[:, core * HD + pidx]),
            "wv": ptile(wv[:, core * HD:(core + 1) * HD]),
            "wo": ptile(wo[:, core * OUTC:(core + 1) * OUTC]),
            "cs": cs,
            "sn": sn,
        })
    return in_maps


def kernel(x, freqs_cos, freqs_sin, wq, wk, wv, wo, _run_kwargs=None):
    in_maps = make_in_maps(x, freqs_cos, freqs_sin, wq, wk, wv, wo)
    nc = _get_nc()
    res = run_bass_kernel_spmd(nc, in_maps, list(range(N_CORES)),
                               **(_run_kwargs or {}))
    out = np.concatenate([res.results[i]["out"] for i in range(N_CORES)], axis=1)
    if _run_kwargs is not None:
        kernel.last_results = res
    return np.ascontiguousarray(out.astype(np.float32))


# revision 18
# speedup vs baseline: 1.2764x; 1.0141x over previous
"""Trainium2 Bass kernel for a GQA causal-attention block (TP over heads, 8 cores).

Computation (per reference): q/k/v projections of x, interleaved RoPE on q/k,
GQA causal attention (32 q heads, 8 kv heads, head_dim 128, seq 2048), output
projection. Sharding: tensor-parallel over heads — each core owns 4 q heads and
their shared kv head. The attention output (transposed layout) is AllGathered
across cores per 512-sequence window and each core computes a 512-column slice
of the final output projection; the host concatenates the column slices.

Device dataflow notes:
  - Everything transposed: x^T streams as the matmul moving operand so q^T/k^T
    come out with head_dim on partitions; scores are computed transposed
    (s^T[k_pos, q_pos]) so exp(s^T) feeds the PV matmul directly as the moving
    operand without any on-chip transposes.
  - RoPE uses a half-split head_dim permutation (evens then odds), folded into
    the wq/wk columns on the host, so the rotation is two 64-partition
    multiply/add pairs against host-transposed cos/sin tables.
  - Softmax skips the max subtraction (scores ~ N(0,1) after scaling); row sums
    come from a ones-matmul accumulated alongside PV; normalization is folded
    into the psum->sbuf eviction of the attention output via a PE-broadcast
    reciprocal row.
  - Causal masking multiplies exp(scores) by a 0/1 mask on diagonal blocks only.
  - Global software pipeline keeps the PE stream dense (no HAM re-throttle):
    QKV(0) runs contraction-outer behind the initial DMAs, then attention
    window qc is emitted right after QKV(qc) (causality makes window qc fully
    computable then), AllGathers fire per window, and each window's output
    projection is emitted several phases later so collective + gather-DMA
    latency hides under guaranteed PE work. Attention windows run in order
    0,1,3,2 so the final output-projection slice has cover for its AllGather.
  - PSUM is exactly 8 banks: window-0 k-outer uses 5 transient banks; steady
    state shares 2 banks between QKV-psum/scores, 2 between v/wo-psum, plus
    2 PV accumulators and 2 softmax-sum accumulators.
"""

import numpy as np
import ml_dtypes

import concourse.bass as bass
import concourse.mybir as mybir
import concourse.tile as tile
from concourse import bacc
from concourse.bass_utils import run_bass_kernel_spmd

N_CORES = 8
P = 128
SEQ = 2048
DIM = 4096
N_HEADS = 32
N_KV_HEADS = 8
HD = 128
QH = N_HEADS // N_CORES        # q heads per core
KD = DIM // P                  # contraction chunks
KG = 4                         # k-chunk DMA groups
KPG = KD // KG                 # k chunks per group
W = 512                        # seq window (matmul moving free dim)
NW = SEQ // W
NT = SEQ // P
OUTC = DIM // N_CORES          # output columns per core
SCALE = HD ** -0.5

BF16 = mybir.dt.bfloat16
F32 = mybir.dt.float32


def _build_nc():
    nc = bacc.Bacc("TRN2", target_bir_lowering=False, debug=False,
                   num_devices=N_CORES)

    xt_d = nc.dram_tensor("xt", [NW, P, KD, W], BF16, kind="ExternalInput")
    wq_d = nc.dram_tensor("wq", [P, KD, QH * HD], BF16, kind="ExternalInput")
    wk_d = nc.dram_tensor("wk", [P, KD, HD], BF16, kind="ExternalInput")
    wv_d = nc.dram_tensor("wv", [P, KD, HD], BF16, kind="ExternalInput")
    wo_d = nc.dram_tensor("wo", [P, KD, OUTC], BF16, kind="ExternalInput")
    cs_d = nc.dram_tensor("cs", [64, SEQ], F32, kind="ExternalInput")
    sn_d = nc.dram_tensor("sn", [64, SEQ], F32, kind="ExternalInput")
    out_d = nc.dram_tensor("out", [SEQ, OUTC], F32, kind="ExternalOutput")

    # 0/1 causal masks for the 4 diagonal alignments of a [128 kv, 512 q] block:
    # mask[p, c, q] = 1 iff kv offset p + c*128 <= q (within the 512-q window).
    j = np.arange(P)[:, None, None]
    c = np.arange(4)[None, :, None]
    q = np.arange(W)[None, None, :]
    masks_np = (j + c * P <= q).astype(ml_dtypes.bfloat16)
    masks_d = nc.inline_tensor(masks_np, "cmasks")
    ones_d = nc.inline_tensor(np.ones([P, P], dtype=ml_dtypes.bfloat16), "ones")
    ones1f_d = nc.inline_tensor(np.ones([1, P], dtype=np.float32), "ones1f")

    Exp = mybir.ActivationFunctionType.Exp

    from contextlib import ExitStack
    with tile.TileContext(nc) as tc:
        with tc.tile_pool(name="const", bufs=1) as constp, \
             tc.tile_pool(name="acts", bufs=1) as actp, \
             tc.tile_pool(name="ropet", bufs=1) as rtp, \
             tc.tile_pool(name="ccdram", bufs=1, space="DRAM") as dcc, \
             tc.tile_pool(name="wop", bufs=1) as wop:
            region_a = ExitStack()
            wp = region_a.enter_context(tc.tile_pool(name="wqkv", bufs=1, side="right"))
            xp = region_a.enter_context(tc.tile_pool(name="xtp", bufs=2, side="right"))
            qrot = actp.tile([P, QH, SEQ], BF16)   # q^T (rope'd), per head
            krot = actp.tile([P, SEQ], BF16)       # k^T (rope'd)
            v_sb = actp.tile([P, NT, HD], BF16)    # v row-tiles [seq, hd]

            wq_g = [wp.tile([P, KPG, QH * HD], BF16, name=f"wq{g}")
                    for g in range(KG)]
            wk_g = [wp.tile([P, KPG, HD], BF16, name=f"wk{g}")
                    for g in range(KG)]
            wv_g = [wp.tile([P, KPG, HD], BF16, name=f"wv{g}")
                    for g in range(KG)]
            xt_gs: dict[tuple[int, int], bass.AP] = {}

            def load_xt_group(w, g):
                t = xp.tile([P, KPG, W], BF16, tag=f"xt{g}")
                eng = nc.sync if g % 2 == 0 else nc.scalar
                eng.dma_start(
                    out=t[:], in_=xt_d[w, :, g * KPG:(g + 1) * KPG, :])
                xt_gs[(w, g)] = t

            def grp(handle, g):
                return handle[:, g * KPG:(g + 1) * KPG, :]

            # initial DMA order: window-0 groups interleaved with weights, then
            # window 1, with the small tables slotted where they're needed.
            for g in range(KG):
                load_xt_group(0, g)
                nc.scalar.dma_start(out=wq_g[g][:], in_=grp(wq_d, g))
                nc.scalar.dma_start(out=wk_g[g][:], in_=grp(wk_d, g))
            load_xt_group(1, 0)
            load_xt_group(1, 1)
            trig_sb = constp.tile([P, SEQ], F32)
            nc.scalar.dma_start(out=trig_sb[0:64, :], in_=cs_d[:])
            nc.scalar.dma_start(out=trig_sb[64:128, :], in_=sn_d[:])
            load_xt_group(1, 2)
            load_xt_group(1, 3)
            for g in range(KG):
                nc.scalar.dma_start(out=wv_g[g][:], in_=grp(wv_d, g))
            masks_sb = constp.tile([P, 4, W], BF16)
            nc.scalar.dma_start(out=masks_sb[:], in_=masks_d[:])
            ones_sb = constp.tile([P, P], BF16)
            nc.scalar.dma_start(out=ones_sb[:], in_=ones_d[:])
            ones1f_sb = constp.tile([1, P], F32)
            nc.scalar.dma_start(out=ones1f_sb[:], in_=ones1f_d[:])
            wo_sb = wop.tile([P, KD, OUTC], BF16)

            ag_in = [dcc.tile([QH * HD, W], BF16, name=f"agin{w}")
                     for w in range(NW)]
            ag_out = [dcc.tile([DIM, W], BF16, addr_space="Shared",
                               name=f"agout{w}")
                      for w in range(NW)]

            def rope(ps, dst, s0):
                cw = trig_sb[0:64, s0:s0 + W]
                sw = trig_sb[64:128, s0:s0 + W]
                t1 = rtp.tile([64, W], F32, tag="t1")
                t2 = rtp.tile([64, W], F32, tag="t2")
                # even half: x1*cos - x2*sin
                nc.vector.tensor_mul(t1[:], ps[0:64, :], cw)
                nc.vector.tensor_mul(t2[:], ps[64:128, :], sw)
                nc.vector.tensor_sub(dst[0:64], t1[:], t2[:])
                # odd half: x1*sin + x2*cos (reuse the t1/t2 slots)
                t3 = rtp.tile([64, W], F32, tag="t1")
                t4 = rtp.tile([64, W], F32, tag="t2")
                nc.vector.tensor_mul(t3[:], ps[0:64, :], sw)
                nc.vector.tensor_mul(t4[:], ps[64:128, :], cw)
                nc.vector.tensor_add(dst[64:128], t3[:], t4[:])

            # window 0 q/k: contraction-outer so PE streams behind the DMAs.
            # Each psum tile owns a full bank (start=True zeroes a whole 2KB
            # zero-region, so accumulation groups must never share a bank).
            with tc.tile_pool(name="ps0", bufs=1, space="PSUM") as p0:
                ps_w0 = [p0.tile([P, W], F32, name=f"ps0_{m}")
                         for m in range(QH + 1)]
                for k in range(KD):
                    g, kk = k // KPG, k % KPG
                    xg = xt_gs[(0, g)]
                    st, sp = (k == 0), (k == KD - 1)
                    for m in range(QH):
                        nc.tensor.matmul(ps_w0[m][:],
                                         wq_g[g][:, kk, m * HD:(m + 1) * HD],
                                         xg[:, kk, :], start=st, stop=sp)
                    nc.tensor.matmul(ps_w0[QH][:], wk_g[g][:, kk, :],
                                     xg[:, kk, :], start=st, stop=sp)
                for m in range(QH):
                    rope(ps_w0[m], qrot[:, m, 0:W], 0)
                rope(ps_w0[QH], krot[:, 0:W], 0)

            # steady-state psum pools: exactly 8 banks, with QKV/scores and
            # v/wo sharing slots (the streams interleave on the PE anyway).
            with tc.tile_pool(name="pmixs", bufs=2, space="PSUM") as pmixs, \
                 tc.tile_pool(name="pmixv", bufs=2, space="PSUM") as pmixv, \
                 tc.tile_pool(name="pso", bufs=2, space="PSUM") as pso, \
                 tc.tile_pool(name="pssum", bufs=2, space="PSUM") as pss, \
                 tc.tile_pool(name="ptp", bufs=3) as ptp, \
                 tc.tile_pool(name="attp", bufs=2) as attp:

                def emit_v(w):
                    for mm in range(W // P):
                        psv = pmixv.tile([P, OUTC], F32, tag="vw")
                        for k in range(KD):
                            g, kk = k // KPG, k % KPG
                            nc.tensor.matmul(
                                psv[:, 0:HD],
                                xt_gs[(w, g)][:, kk, mm * P:(mm + 1) * P],
                                wv_g[g][:, kk, :],
                                start=(k == 0), stop=(k == KD - 1))
                        nc.scalar.copy(v_sb[:, w * (W // P) + mm, :],
                                       psv[:, 0:HD])

                def emit_qkv(w):
                    s0 = w * W
                    if w + 1 < NW:
                        for g in range(KG):
                            load_xt_group(w + 1, g)
                    for m in range(QH + 1):
                        ps = pmixs.tile([P, W], F32, tag="s")
                        for k in range(KD):
                            g, kk = k // KPG, k % KPG
                            lhsT = (wq_g[g][:, kk, m * HD:(m + 1) * HD]
                                    if m < QH else wk_g[g][:, kk, :])
                            nc.tensor.matmul(ps[:], lhsT,
                                             xt_gs[(w, g)][:, kk, :],
                                             start=(k == 0), stop=(k == KD - 1))
                        dst = (qrot[:, m, s0:s0 + W] if m < QH
                               else krot[:, s0:s0 + W])
                        rope(ps, dst, s0)
                    emit_v(w)

                def emit_attention(qc):
                    q0 = qc * W
                    nkv = (W // P) * (qc + 1)

                    def finish(ps_o, ps_sum, h):
                        # normalization: reciprocal of one row, PE-broadcast to
                        # 128 partitions, multiply into the psum eviction. The
                        # reciprocal was issued a head earlier, so the PE never
                        # waits on it.
                        ps_bc = pmixs.tile([P, W], F32, tag="s")
                        nc.tensor.matmul(ps_bc[:], ones1f_sb[:], invs[h][:],
                                         start=True, stop=True)
                        inv_bc = attp.tile([P, W], F32, tag="invbc")
                        nc.vector.tensor_copy(inv_bc[:], ps_bc[:])
                        at = attp.tile([P, W], BF16, tag="at")
                        nc.vector.tensor_mul(at[:], ps_o[:], inv_bc[:])
                        nc.sync.dma_start(out=ag_in[qc][h * HD:(h + 1) * HD, :],
                                          in_=at[:])

                    invs = {}
                    pend = None
                    for h in range(QH):
                        ps_o = pso.tile([P, W], F32, tag="o")
                        ps_sum = pss.tile([P, W], F32, tag="sum")
                        for jj in range(nkv):
                            j0 = jj * P
                            ps_s = pmixs.tile([P, W], F32, tag="s")
                            nc.tensor.matmul(ps_s[:], krot[:, j0:j0 + P],
                                             qrot[:, h, q0:q0 + W],
                                             start=True, stop=True)
                            pt = ptp.tile([P, W], BF16, tag="pt")
                            nc.scalar.activation(pt[:], ps_s[:], Exp, scale=SCALE)
                            if jj >= (W // P) * qc:
                                nc.vector.tensor_mul(
                                    pt[:], pt[:],
                                    masks_sb[:, jj - (W // P) * qc, :])
                            nc.tensor.matmul(ps_o[:], v_sb[:, jj, :], pt[:],
                                             start=(jj == 0), stop=(jj == nkv - 1))
                            nc.tensor.matmul(ps_sum[:], ones_sb[:], pt[:],
                                             start=(jj == 0), stop=(jj == nkv - 1))
                        inv1 = attp.tile([1, W], F32, tag="inv1")
                        nc.vector.reciprocal_approx_fast(out=inv1[:], in_=ps_sum[0:1, :])
                        invs[h] = inv1
                        if pend is not None:
                            finish(*pend)
                        pend = (ps_o, ps_sum, h)
                    finish(*pend)
                    nc.gpsimd.collective_compute(
                        "AllGather", mybir.AluOpType.bypass,
                        replica_groups=[list(range(N_CORES))],
                        ins=[ag_in[qc][:]], outs=[ag_out[qc][:]])

                atfs: dict[int, bass.AP] = {}

                def load_atf(qc):
                    atf = atfp.tile([P, KD, W], BF16, tag="atf")
                    nc.sync.dma_start(
                        out=atf[:],
                        in_=ag_out[qc][:].rearrange("(k p) n -> p k n", p=P))
                    atfs[qc] = atf

                def emit_wo(qc):
                    q0 = qc * W
                    atf = atfs.pop(qc)
                    for mm in range(W // P):
                        m0 = mm * P
                        ps = pmixv.tile([P, OUTC], F32, tag="vw")
                        for k in range(KD):
                            nc.tensor.matmul(ps[:],
                                             atf[:, k, m0:m0 + P],
                                             wo_sb[:, k, :],
                                             start=(k == 0), stop=(k == KD - 1))
                        ot = outp.tile([P, OUTC], F32, tag="ot")
                        nc.vector.tensor_copy(ot[:], ps[:])
                        nc.scalar.dma_start(out=out_d[q0 + m0:q0 + m0 + P, :],
                                            in_=ot[:])

                emit_v(0)
                emit_attention(0)
                emit_qkv(1)
                emit_attention(1)
                emit_qkv(2)
                nc.scalar.dma_start(out=wo_sb[:], in_=wo_d[:])
                emit_qkv(3)
                region_a.close()   # frees the weight/x^T pools for atf staging
                region_b = ExitStack()
                atfp = region_b.enter_context(tc.tile_pool(name="atfp", bufs=3, side="right"))
                outp = region_b.enter_context(tc.tile_pool(name="outp", bufs=3, side="right"))
                load_atf(0)
                load_atf(1)
                emit_attention(3)
                emit_attention(2)
                load_atf(3)
                emit_wo(0)
                emit_wo(1)
                load_atf(2)
                emit_wo(3)
                emit_wo(2)
                region_b.close()

    nc.compile()
    return nc


_NC_CACHE = None


def _get_nc():
    global _NC_CACHE
    if _NC_CACHE is None:
        _NC_CACHE = _build_nc()
    return _NC_CACHE


def make_in_maps(x, freqs_cos, freqs_sin, wq, wk, wv, wo):
    bf16 = ml_dtypes.bfloat16
    # half-split permutation: evens then odds within each head's 128 dims
    pidx = np.concatenate([np.arange(0, HD, 2), np.arange(1, HD, 2)])

    def ptile(w):
        # [DIM, cols] -> [P, KD, cols] so per-partition DMA runs are contiguous
        return np.ascontiguousarray(
            w.reshape(KD, P, w.shape[1]).transpose(1, 0, 2).astype(bf16))

    xt = np.ascontiguousarray(
        np.asarray(x).T.astype(bf16).reshape(KD, P, NW, W).transpose(2, 1, 0, 3))
    cs = np.ascontiguousarray(np.asarray(freqs_cos).T.astype(np.float32))
    sn = np.ascontiguousarray(np.asarray(freqs_sin).T.astype(np.float32))
    wq = np.asarray(wq)
    wk = np.asarray(wk)
    wv = np.asarray(wv)
    wo = np.asarray(wo)
    in_maps = []
    for core in range(N_CORES):
        q_cols = np.concatenate([h * HD + pidx
                                 for h in range(QH * core, QH * (core + 1))])
        in_maps.append({
            "xt": xt,
            "wq": ptile(wq[:, q_cols]),
            "wk": ptile(wk[:, core * HD + pidx]),
            "wv": ptile(wv[:, core * HD:(core + 1) * HD]),
            "wo": ptile(wo[:, core * OUTC:(core + 1) * OUTC]),
            "cs": cs,
            "sn": sn,
        })
    return in_maps


def kernel(x, freqs_cos, freqs_sin, wq, wk, wv, wo, _run_kwargs=None):
    in_maps = make_in_maps(x, freqs_cos, freqs_sin, wq, wk, wv, wo)
    nc = _get_nc()
    res = run_bass_kernel_spmd(nc, in_maps, list(range(N_CORES)),
                               **(_run_kwargs or {}))
    out = np.concatenate([res.results[i]["out"] for i in range(N_CORES)], axis=1)
    if _run_kwargs is not None:
        kernel.last_results = res
    return np.ascontiguousarray(out.astype(np.float32))


# revision 19
# speedup vs baseline: 1.3264x; 1.0392x over previous
"""Trainium2 Bass kernel for a GQA causal-attention block (TP over heads, 8 cores).

Computation (per reference): q/k/v projections of x, interleaved RoPE on q/k,
GQA causal attention (32 q heads, 8 kv heads, head_dim 128, seq 2048), output
projection. Sharding: tensor-parallel over heads — each core owns 4 q heads and
their shared kv head. The attention output (transposed layout) is AllGathered
across cores per 512-sequence window and each core computes a 512-column slice
of the final output projection; the host concatenates the column slices.

Device dataflow notes:
  - Everything transposed: x^T streams as the matmul moving operand so q^T/k^T
    come out with head_dim on partitions; scores are computed transposed
    (s^T[k_pos, q_pos]) so exp(s^T) feeds the PV matmul directly as the moving
    operand without any on-chip transposes.
  - RoPE uses a half-split head_dim permutation (evens then odds), folded into
    the wq/wk columns on the host, so the rotation is two 64-partition
    multiply/add pairs against host-transposed cos/sin tables.
  - Softmax skips the max subtraction (scores ~ N(0,1) after scaling); row sums
    come from a ones-matmul accumulated alongside PV; normalization is folded
    into the psum->sbuf eviction of the attention output via a PE-broadcast
    reciprocal row.
  - Causal masking multiplies exp(scores) by a 0/1 mask on diagonal blocks only.
  - Global software pipeline keeps the PE stream dense (no HAM re-throttle):
    QKV(0) runs contraction-outer behind the initial DMAs, then attention
    window qc is emitted right after QKV(qc) (causality makes window qc fully
    computable then), AllGathers fire per window, and each window's output
    projection is emitted several phases later so collective + gather-DMA
    latency hides under guaranteed PE work. Attention windows run in order
    0,1,3,2 so the final output-projection slice has cover for its AllGather.
  - PSUM is exactly 8 banks: window-0 k-outer uses 5 transient banks; steady
    state shares 2 banks between QKV-psum/scores, 2 between v/wo-psum, plus
    2 PV accumulators and 2 softmax-sum accumulators.
"""

import numpy as np
import ml_dtypes

import concourse.bass as bass
import concourse.mybir as mybir
import concourse.tile as tile
from concourse import bacc
from concourse.bass_utils import run_bass_kernel_spmd

N_CORES = 8
P = 128
SEQ = 2048
DIM = 4096
N_HEADS = 32
N_KV_HEADS = 8
HD = 128
QH = N_HEADS // N_CORES        # q heads per core
KD = DIM // P                  # contraction chunks
KG = 4                         # k-chunk DMA groups
KPG = KD // KG                 # k chunks per group
W = 512                        # seq window (matmul moving free dim)
NW = SEQ // W
NT = SEQ // P
OUTC = DIM // N_CORES          # output columns per core
SCALE = HD ** -0.5

BF16 = mybir.dt.bfloat16
F32 = mybir.dt.float32


def _build_nc():
    nc = bacc.Bacc("TRN2", target_bir_lowering=False, debug=False,
                   num_devices=N_CORES)

    xt_d = nc.dram_tensor("xt", [NW, P, KD, W], BF16, kind="ExternalInput")
    wq_d = nc.dram_tensor("wq", [P, KD, QH * HD], BF16, kind="ExternalInput")
    wk_d = nc.dram_tensor("wk", [P, KD, HD], BF16, kind="ExternalInput")
    wv_d = nc.dram_tensor("wv", [P, KD, HD], BF16, kind="ExternalInput")
    wo_d = nc.dram_tensor("wo", [P, KD, OUTC], BF16, kind="ExternalInput")
    cs_d = nc.dram_tensor("cs", [64, SEQ], F32, kind="ExternalInput")
    sn_d = nc.dram_tensor("sn", [64, SEQ], F32, kind="ExternalInput")
    out_d = nc.dram_tensor("out", [SEQ, OUTC], F32, kind="ExternalOutput")

    # 0/1 causal masks for the 4 diagonal alignments of a [128 kv, 512 q] block:
    # mask[p, c, q] = 1 iff kv offset p + c*128 <= q (within the 512-q window).
    j = np.arange(P)[:, None, None]
    c = np.arange(4)[None, :, None]
    q = np.arange(W)[None, None, :]
    masks_np = (j + c * P <= q).astype(ml_dtypes.bfloat16)
    masks_d = nc.inline_tensor(masks_np, "cmasks")
    ones_d = nc.inline_tensor(np.ones([P, P], dtype=ml_dtypes.bfloat16), "ones")
    ones1f_d = nc.inline_tensor(np.ones([1, P], dtype=np.float32), "ones1f")

    Exp = mybir.ActivationFunctionType.Exp

    from contextlib import ExitStack
    with tile.TileContext(nc) as tc:
        with tc.tile_pool(name="const", bufs=1) as constp, \
             tc.tile_pool(name="acts", bufs=1) as actp, \
             tc.tile_pool(name="ropet", bufs=1) as rtp, \
             tc.tile_pool(name="ccdram", bufs=1, space="DRAM") as dcc, \
             tc.tile_pool(name="wop", bufs=1) as wop:
            region_a = ExitStack()
            wp = region_a.enter_context(tc.tile_pool(name="wqkv", bufs=1, side="right"))
            xp = region_a.enter_context(tc.tile_pool(name="xtp", bufs=2, side="right"))
            qrot = actp.tile([P, QH, SEQ], BF16)   # q^T (rope'd), per head
            krot = actp.tile([P, SEQ], BF16)       # k^T (rope'd)
            v_sb = actp.tile([P, NT, HD], BF16)    # v row-tiles [seq, hd]

            wq_g = [wp.tile([P, KPG, QH * HD], BF16, name=f"wq{g}")
                    for g in range(KG)]
            wk_g = [wp.tile([P, KPG, HD], BF16, name=f"wk{g}")
                    for g in range(KG)]
            wv_g = [wp.tile([P, KPG, HD], BF16, name=f"wv{g}")
                    for g in range(KG)]
            xt_gs: dict[tuple[int, int], bass.AP] = {}

            def load_xt_group(w, g):
                t = xp.tile([P, KPG, W], BF16, tag=f"xt{g}")
                eng = (nc.sync, nc.scalar, nc.gpsimd)[g % 3] if w < 2 \
                    else (nc.sync if g % 2 == 0 else nc.scalar)
                eng.dma_start(
                    out=t[:], in_=xt_d[w, :, g * KPG:(g + 1) * KPG, :])
                xt_gs[(w, g)] = t

            def grp(handle, g):
                return handle[:, g * KPG:(g + 1) * KPG, :]

            # initial DMA order: window-0 groups interleaved with weights, then
            # window 1, with the small tables slotted where they're needed.
            for g in range(KG):
                load_xt_group(0, g)
                nc.scalar.dma_start(out=wq_g[g][:], in_=grp(wq_d, g))
                nc.scalar.dma_start(out=wk_g[g][:], in_=grp(wk_d, g))
            load_xt_group(1, 0)
            load_xt_group(1, 1)
            trig_sb = constp.tile([P, SEQ], F32)
            nc.scalar.dma_start(out=trig_sb[0:64, :], in_=cs_d[:])
            nc.scalar.dma_start(out=trig_sb[64:128, :], in_=sn_d[:])
            load_xt_group(1, 2)
            load_xt_group(1, 3)
            for g in range(KG):
                nc.scalar.dma_start(out=wv_g[g][:], in_=grp(wv_d, g))
            masks_sb = constp.tile([P, 4, W], BF16)
            nc.scalar.dma_start(out=masks_sb[:], in_=masks_d[:])
            ones_sb = constp.tile([P, P], BF16)
            nc.scalar.dma_start(out=ones_sb[:], in_=ones_d[:])
            ones1f_sb = constp.tile([1, P], F32)
            nc.scalar.dma_start(out=ones1f_sb[:], in_=ones1f_d[:])
            wo_sb = wop.tile([P, KD, OUTC], BF16)

            ag_in = [dcc.tile([QH * HD, W], BF16, name=f"agin{w}")
                     for w in range(NW)]
            ag_out = [dcc.tile([DIM, W], BF16, addr_space="Shared",
                               name=f"agout{w}")
                      for w in range(NW)]

            def rope(ps, dst, s0):
                cw = trig_sb[0:64, s0:s0 + W]
                sw = trig_sb[64:128, s0:s0 + W]
                t1 = rtp.tile([64, W], F32, tag="t1")
                t2 = rtp.tile([64, W], F32, tag="t2")
                # even half: x1*cos - x2*sin
                nc.vector.tensor_mul(t1[:], ps[0:64, :], cw)
                nc.vector.tensor_mul(t2[:], ps[64:128, :], sw)
                nc.vector.tensor_sub(dst[0:64], t1[:], t2[:])
                # odd half: x1*sin + x2*cos (reuse the t1/t2 slots)
                t3 = rtp.tile([64, W], F32, tag="t1")
                t4 = rtp.tile([64, W], F32, tag="t2")
                nc.vector.tensor_mul(t3[:], ps[0:64, :], sw)
                nc.vector.tensor_mul(t4[:], ps[64:128, :], cw)
                nc.vector.tensor_add(dst[64:128], t3[:], t4[:])

            # window 0 q/k: contraction-outer so PE streams behind the DMAs.
            # Each psum tile owns a full bank (start=True zeroes a whole 2KB
            # zero-region, so accumulation groups must never share a bank).
            with tc.tile_pool(name="ps0", bufs=1, space="PSUM") as p0:
                ps_w0 = [p0.tile([P, W], F32, name=f"ps0_{m}")
                         for m in range(QH + 1)]
                for k in range(KD):
                    g, kk = k // KPG, k % KPG
                    xg = xt_gs[(0, g)]
                    st, sp = (k == 0), (k == KD - 1)
                    for m in range(QH):
                        nc.tensor.matmul(ps_w0[m][:],
                                         wq_g[g][:, kk, m * HD:(m + 1) * HD],
                                         xg[:, kk, :], start=st, stop=sp)
                    nc.tensor.matmul(ps_w0[QH][:], wk_g[g][:, kk, :],
                                     xg[:, kk, :], start=st, stop=sp)
                for m in range(QH):
                    rope(ps_w0[m], qrot[:, m, 0:W], 0)
                rope(ps_w0[QH], krot[:, 0:W], 0)

            # steady-state psum pools: exactly 8 banks, with QKV/scores and
            # v/wo sharing slots (the streams interleave on the PE anyway).
            with tc.tile_pool(name="pmixs", bufs=2, space="PSUM") as pmixs, \
                 tc.tile_pool(name="pmixv", bufs=2, space="PSUM") as pmixv, \
                 tc.tile_pool(name="pso", bufs=2, space="PSUM") as pso, \
                 tc.tile_pool(name="pssum", bufs=2, space="PSUM") as pss, \
                 tc.tile_pool(name="ptp", bufs=3) as ptp, \
                 tc.tile_pool(name="attp", bufs=2) as attp:

                def emit_v(w):
                    for mm in range(W // P):
                        psv = pmixv.tile([P, OUTC], F32, tag="vw")
                        for k in range(KD):
                            g, kk = k // KPG, k % KPG
                            nc.tensor.matmul(
                                psv[:, 0:HD],
                                xt_gs[(w, g)][:, kk, mm * P:(mm + 1) * P],
                                wv_g[g][:, kk, :],
                                start=(k == 0), stop=(k == KD - 1))
                        nc.scalar.copy(v_sb[:, w * (W // P) + mm, :],
                                       psv[:, 0:HD])

                def emit_qkv(w):
                    s0 = w * W
                    if w + 1 < NW:
                        for g in range(KG):
                            load_xt_group(w + 1, g)
                    for m in range(QH + 1):
                        ps = pmixs.tile([P, W], F32, tag="s")
                        for k in range(KD):
                            g, kk = k // KPG, k % KPG
                            lhsT = (wq_g[g][:, kk, m * HD:(m + 1) * HD]
                                    if m < QH else wk_g[g][:, kk, :])
                            nc.tensor.matmul(ps[:], lhsT,
                                             xt_gs[(w, g)][:, kk, :],
                                             start=(k == 0), stop=(k == KD - 1))
                        dst = (qrot[:, m, s0:s0 + W] if m < QH
                               else krot[:, s0:s0 + W])
                        rope(ps, dst, s0)
                    emit_v(w)

                def emit_attention(qc):
                    q0 = qc * W
                    nkv = (W // P) * (qc + 1)

                    def finish(ps_o, ps_sum, h):
                        # normalization: reciprocal of one row, PE-broadcast to
                        # 128 partitions, multiply into the psum eviction. The
                        # reciprocal was issued a head earlier, so the PE never
                        # waits on it.
                        ps_bc = pmixs.tile([P, W], F32, tag="s")
                        nc.tensor.matmul(ps_bc[:], ones1f_sb[:], invs[h][:],
                                         start=True, stop=True)
                        inv_bc = attp.tile([P, W], F32, tag="invbc")
                        nc.vector.tensor_copy(inv_bc[:], ps_bc[:])
                        at = attp.tile([P, W], BF16, tag="at", bufs=3)
                        nc.vector.tensor_mul(at[:], ps_o[:], inv_bc[:])
                        nc.sync.dma_start(out=ag_in[qc][h * HD:(h + 1) * HD, :],
                                          in_=at[:])

                    invs = {}
                    pend = None
                    for h in range(QH):
                        ps_o = pso.tile([P, W], F32, tag="o")
                        ps_sum = pss.tile([P, W], F32, tag="sum")
                        for jj in range(nkv):
                            j0 = jj * P
                            ps_s = pmixs.tile([P, W], F32, tag="s")
                            nc.tensor.matmul(ps_s[:], krot[:, j0:j0 + P],
                                             qrot[:, h, q0:q0 + W],
                                             start=True, stop=True)
                            pt = ptp.tile([P, W], BF16, tag="pt")
                            nc.scalar.activation(pt[:], ps_s[:], Exp, scale=SCALE)
                            if jj >= (W // P) * qc:
                                nc.vector.tensor_mul(
                                    pt[:], pt[:],
                                    masks_sb[:, jj - (W // P) * qc, :])
                            nc.tensor.matmul(ps_o[:], v_sb[:, jj, :], pt[:],
                                             start=(jj == 0), stop=(jj == nkv - 1))
                            nc.tensor.matmul(ps_sum[:], ones_sb[:], pt[:],
                                             start=(jj == 0), stop=(jj == nkv - 1))
                        inv1 = attp.tile([1, W], F32, tag="inv1")
                        nc.vector.reciprocal_approx_fast(out=inv1[:], in_=ps_sum[0:1, :])
                        invs[h] = inv1
                        if pend is not None:
                            finish(*pend)
                        pend = (ps_o, ps_sum, h)
                    finish(*pend)
                    nc.gpsimd.collective_compute(
                        "AllGather", mybir.AluOpType.bypass,
                        replica_groups=[list(range(N_CORES))],
                        ins=[ag_in[qc][:]], outs=[ag_out[qc][:]])

                atfs: dict[int, bass.AP] = {}

                def load_atf(qc):
                    atf = atfp.tile([P, KD, W], BF16, tag="atf")
                    nc.gpsimd.dma_start(
                        out=atf[:],
                        in_=ag_out[qc][:].rearrange("(k p) n -> p k n", p=P))
                    atfs[qc] = atf

                def emit_wo(qc):
                    q0 = qc * W
                    atf = atfs.pop(qc)
                    for mm in range(W // P):
                        m0 = mm * P
                        ps = pmixv.tile([P, OUTC], F32, tag="vw")
                        for k in range(KD):
                            nc.tensor.matmul(ps[:],
                                             atf[:, k, m0:m0 + P],
                                             wo_sb[:, k, :],
                                             start=(k == 0), stop=(k == KD - 1))
                        ot = outp.tile([P, OUTC], F32, tag="ot")
                        nc.vector.tensor_copy(ot[:], ps[:])
                        nc.scalar.dma_start(out=out_d[q0 + m0:q0 + m0 + P, :],
                                            in_=ot[:])

                emit_v(0)
                emit_attention(0)
                emit_qkv(1)
                emit_attention(1)
                emit_qkv(2)
                nc.scalar.dma_start(out=wo_sb[:], in_=wo_d[:])
                emit_qkv(3)
                region_a.close()   # frees the weight/x^T pools for atf staging
                region_b = ExitStack()
                atfp = region_b.enter_context(tc.tile_pool(name="atfp", bufs=3, side="right"))
                outp = region_b.enter_context(tc.tile_pool(name="outp", bufs=3, side="right"))
                load_atf(0)
                load_atf(1)
                emit_attention(3)
                emit_attention(2)
                load_atf(3)
                emit_wo(0)
                emit_wo(1)
                load_atf(2)
                emit_wo(3)
                emit_wo(2)
                region_b.close()

    nc.compile()
    return nc


_NC_CACHE = None


def _get_nc():
    global _NC_CACHE
    if _NC_CACHE is None:
        _NC_CACHE = _build_nc()
    return _NC_CACHE


def make_in_maps(x, freqs_cos, freqs_sin, wq, wk, wv, wo):
    bf16 = ml_dtypes.bfloat16
    # half-split permutation: evens then odds within each head's 128 dims
    pidx = np.concatenate([np.arange(0, HD, 2), np.arange(1, HD, 2)])

    def ptile(w):
        # [DIM, cols] -> [P, KD, cols] so per-partition DMA runs are contiguous
        return np.ascontiguousarray(
            w.reshape(KD, P, w.shape[1]).transpose(1, 0, 2).astype(bf16))

    xt = np.ascontiguousarray(
        np.asarray(x).T.astype(bf16).reshape(KD, P, NW, W).transpose(2, 1, 0, 3))
    cs = np.ascontiguousarray(np.asarray(freqs_cos).T.astype(np.float32))
    sn = np.ascontiguousarray(np.asarray(freqs_sin).T.astype(np.float32))
    wq = np.asarray(wq)
    wk = np.asarray(wk)
    wv = np.asarray(wv)
    wo = np.asarray(wo)
    in_maps = []
    for core in range(N_CORES):
        q_cols = np.concatenate([h * HD + pidx
                                 for h in range(QH * core, QH * (core + 1))])
        in_maps.append({
            "xt": xt,
            "wq": ptile(wq[:, q_cols]),
            "wk": ptile(wk[:, core * HD + pidx]),
            "wv": ptile(wv[:, core * HD:(core + 1) * HD]),
            "wo": ptile(wo[:, core * OUTC:(core + 1) * OUTC]),
            "cs": cs,
            "sn": sn,
        })
    return in_maps


def kernel(x, freqs_cos, freqs_sin, wq, wk, wv, wo, _run_kwargs=None):
    in_maps = make_in_maps(x, freqs_cos, freqs_sin, wq, wk, wv, wo)
    nc = _get_nc()
    res = run_bass_kernel_spmd(nc, in_maps, list(range(N_CORES)),
                               **(_run_kwargs or {}))
    out = np.concatenate([res.results[i]["out"] for i in range(N_CORES)], axis=1)
    if _run_kwargs is not None:
        kernel.last_results = res
    return np.ascontiguousarray(out.astype(np.float32))
